# revision 1
# baseline (speedup 1.0000x reference)
"""Multi-head attention (q/k/v projections + softmax attention + out-projection)
on 8 Trainium2 NeuronCores.

Sharding: 16 (batch, head) units over 8 cores -> core c handles batch n = c//4
and head pair hp = c%4 (columns 128*hp : 128*hp+128 of the projections).
Per-core partial outputs (each pair's contribution to mix @ Wo) are summed on
host per batch, + bo.

Device kernel (per core):
  - Host pre-transposes q[n],k[n],v[n] -> xT [512, 4096] so the D-contraction
    projections need no on-device transpose.
  - All matmuls run in float32r (TF32-like, 1 cycle/row vs 4 for fp32 on the
    PE; measured end-to-end |err|_max/|out|_max ~ 5e-4 vs fp32's 3e-6).
  - QPT/KPT [128, 512]x8 chunk tiles: W.T @ x with head-dim on partitions;
    q scaled by 1/8, biases folded in via DVE tensor_scalar(mult, add).
  - VP chunk tiles [128lkv, 4, 130]: v-projection computed un-transposed (lkv
    on partitions) with the bias added via an extra K=1 ones x bv matmul row;
    layout per j: [h0 c(64) | ones | h1 c(64) | ones] - the ones column makes
    each PV matmul also accumulate sum(exp) into psum row 64.
  - Attention in S^T orientation: S^T[lkv,lq] = KPT_h.T @ QPT_h (K=64, heads
    at PE row groups 0/64), exp on ScalarE with FD=1024 tiles (no max
    subtraction needed: scores ~ N(0,1), exp range is tiny), PV accumulates
    mixT[c,lq] + sumexp transpose-free. ScalarE exp (~270us busy) is the
    critical engine; S/PV matmuls (~220us) hide under it.
  - sumexp [1,1024] is transposed to [128,8] partition-major via a DRAM
    bounce, reciprocal on DVE, and the normalization is folded into the
    out-projection: per-head out-proj psums scaled per-partition (lq) by
    1/sumexp on DVE, then summed on GPSIMD.
  - One unified 8-bank PSUM pool (st0/st1/pv0/pv1 tags x 2 banks); the
    projection phase rotates through the same tags.
"""

import numpy as np

import concourse.bacc as bacc
import concourse.mybir as mybir
import concourse.tile as tile
from concourse import bass_utils

P = 128
L = 4096
D = 512
F32 = mybir.dt.float32
F32R_DT = mybir.dt.float32r
AF = mybir.ActivationFunctionType

_NC = None
F32R = True  # run matmuls in float32r (TF32-like, 4x faster than fp32 on PE)


def _mm(nc, out, lhsT, rhs, f32r=True, **kw):
    nc.tensor.matmul(out, lhsT=lhsT, rhs=rhs, **kw)


def build():
    nc = bacc.Bacc("TRN2", target_bir_lowering=False, debug=False)

    xqt = nc.dram_tensor("xqt", (D, L), F32R_DT, kind="ExternalInput").ap()
    xkt = nc.dram_tensor("xkt", (D, L), F32R_DT, kind="ExternalInput").ap()
    xvt = nc.dram_tensor("xvt", (D, L), F32R_DT, kind="ExternalInput").ap()
    wq = nc.dram_tensor("wq", (D, P), F32R_DT, kind="ExternalInput").ap()
    wk = nc.dram_tensor("wk", (D, P), F32R_DT, kind="ExternalInput").ap()
    wv = nc.dram_tensor("wv", (D, P), F32R_DT, kind="ExternalInput").ap()
    wo = nc.dram_tensor("wo", (P, D), F32R_DT, kind="ExternalInput").ap()
    bqs = nc.dram_tensor("bqs", (P, 1), F32, kind="ExternalInput").ap()
    bkc = nc.dram_tensor("bkc", (P, 1), F32, kind="ExternalInput").ap()
    bvr = nc.dram_tensor("bvr", (1, P), F32R_DT, kind="ExternalInput").ap()
    out = nc.dram_tensor("out", (L, D), F32, kind="ExternalOutput").ap()

    with tile.TileContext(nc) as tc:
        with tc.tile_pool(name="const", bufs=1) as const, \
             tc.tile_pool(name="persist", bufs=1) as persist:
            wq_sb = const.tile([P, 4, P], F32R_DT, tag="wq")
            nc.sync.dma_start(wq_sb, wq.rearrange("(o p) m -> p o m", p=P))
            wk_sb = const.tile([P, 4, P], F32R_DT, tag="wk")
            nc.sync.dma_start(wk_sb, wk.rearrange("(o p) m -> p o m", p=P))
            wv_sb = const.tile([P, 4, P], F32R_DT, tag="wv")
            nc.sync.dma_start(wv_sb, wv.rearrange("(o p) m -> p o m", p=P))
            wo_sb = const.tile([P, D], F32R_DT, tag="wo")
            nc.sync.dma_start(wo_sb, wo)
            bq_sb = const.tile([P, 1], F32, tag="bq")
            nc.sync.dma_start(bq_sb, bqs)
            bk_sb = const.tile([P, 1], F32, tag="bk")
            nc.sync.dma_start(bk_sb, bkc)
            bvr_sb = const.tile([1, P], F32R_DT, tag="bvr")
            nc.sync.dma_start(bvr_sb, bvr)
            onesr = const.tile([1, P], F32R_DT, tag="onesr")
            nc.scalar.activation(onesr, bvr_sb, AF.Identity,
                                 bias=1.0, scale=0.0)

            qpt_t = [persist.tile([P, 512], F32R_DT, tag=f"qpt{c}",
                                  name=f"qpt{c}") for c in range(8)]
            kpt_t = [persist.tile([P, 512], F32R_DT, tag=f"kpt{c}",
                                  name=f"kpt{c}") for c in range(8)]
            vp_t = [persist.tile([P, 4, 130], F32R_DT, tag=f"vp{c}",
                                 name=f"vp{c}") for c in range(8)]
            ones_in = bq_sb[:, :, None].to_broadcast((P, 4, 1))
            for c in range(8):
                nc.scalar.activation(vp_t[c][:, :, 64:65], ones_in,
                                     AF.Identity, bias=1.0, scale=0.0)
                nc.scalar.activation(vp_t[c][:, :, 129:130], ones_in,
                                     AF.Identity, bias=1.0, scale=0.0)

            # ---------------- projections + attention ----------------
            # One PSUM pool for everything (8 banks exactly): st0/st1 and
            # pv0/pv1 tags, [128,1024] fp32 = 2 banks each. Projections
            # rotate through the same tags so attention tiles never wait on
            # a disjoint pool's address range.
            with tc.tile_pool(name="xs", bufs=2) as xs, \
                 tc.tile_pool(name="psp", bufs=1, space="PSUM") as psp, \
                 tc.tile_pool(name="esp", bufs=4) as esp, \
                 tc.tile_pool(name="smallp", bufs=4) as smallp, \
                 tc.tile_pool(name="mixp", bufs=4) as mixp, \
                 tc.tile_pool(name="outp", bufs=3) as outp, \
                 tc.tile_pool(name="dramp", bufs=2, space="DRAM") as dramp:
                xqv = xqt.rearrange("(o p) l -> p o l", p=P)
                xkv = xkt.rearrange("(o p) l -> p o l", p=P)
                xvv = xvt.rearrange("(o p) l -> p o l", p=P)
                for ch in range(8):
                    sl = slice(ch * 512, (ch + 1) * 512)
                    # K chunk
                    xtk = xs.tile([P, 4, 512], F32R_DT, tag="xtk")
                    nc.sync.dma_start(xtk, xkv[:, :, sl])
                    ps = psp.tile([P, 512], F32, tag="st0", name="kps")
                    for dk in range(4):
                        _mm(nc, ps, wk_sb[:, dk, :], xtk[:, dk, :],
                            start=(dk == 0), stop=(dk == 3))
                    nc.vector.tensor_scalar(
                        kpt_t[ch][:], ps, 1.0, bk_sb,
                        mybir.AluOpType.mult, mybir.AluOpType.add)
                    # V chunk
                    xtv = xs.tile([P, 4, 512], F32R_DT, tag="xtv")
                    nc.gpsimd.dma_start(xtv, xvv[:, :, sl])
                    for js in range(4):
                        j = ch * 4 + js
                        psv = psp.tile([P, P], F32, tag=f"pv{js % 2}",
                                       name="psv")
                        for dk in range(4):
                            _mm(nc, psv, xtv[:, dk, js * P:(js + 1) * P],
                                wv_sb[:, dk, :],
                                start=(dk == 0), stop=False)
                        _mm(nc, psv, onesr, bvr_sb,
                            start=False, stop=True)
                        nc.vector.tensor_copy(vp_t[ch][:, js, 0:64],
                                               psv[:, 0:64])
                        nc.vector.tensor_copy(vp_t[ch][:, js, 65:129],
                                              psv[:, 64:128])
                    # Q chunk
                    xtq = xs.tile([P, 4, 512], F32R_DT, tag="xtq")
                    nc.gpsimd.dma_start(xtq, xqv[:, :, sl])
                    psq = psp.tile([P, 512], F32, tag="st1", name="qps")
                    for dk in range(4):
                        _mm(nc, psq, wq_sb[:, dk, :], xtq[:, dk, :],
                            start=(dk == 0), stop=(dk == 3))
                    nc.vector.tensor_scalar(
                        qpt_t[ch][:], psq, 0.125, bq_sb,
                        mybir.AluOpType.mult, mybir.AluOpType.add)

                # ---------------- attention ----------------
                for lqc in range(4):
                    q0 = lqc * 1024
                    pv_ps = [psp.tile([P, 1024], F32, tag=f"pv{h}", name=f"pv{h}")
                             for h in range(2)]
                    for j in range(32):
                        for h in range(2):
                            hb = h * 64
                            st = psp.tile([P, 1024], F32, tag=f"st{h}")
                            for hf in range(2):
                                _mm(nc, st[:, hf * 512:(hf + 1) * 512],
                                    kpt_t[j // 4][hb:hb + 64,
                                                  (j % 4) * P:(j % 4 + 1) * P],
                                    qpt_t[2 * lqc + hf][hb:hb + 64, :],
                                    start=True, stop=True)
                            est = esp.tile([P, 1024], F32R_DT, tag=f"est{h}")
                            nc.scalar.activation(est, st, AF.Exp)
                            for hf in range(2):
                                _mm(nc, pv_ps[h][0:65, hf * 512:(hf + 1) * 512],
                                    vp_t[j // 4][:, j % 4, h * 65:(h + 1) * 65],
                                    est[:, hf * 512:(hf + 1) * 512],
                                    start=(j == 0), stop=(j == 31))
                    recips = []
                    mix2 = mixp.tile([P, 1024], F32R_DT, tag="mix2")
                    for h in range(2):
                        row = smallp.tile([1, 1024], F32, tag=f"row{h}")
                        nc.vector.tensor_copy(row, pv_ps[h][64:65, :])
                        drow = dramp.tile([1024], F32, tag=f"drow{h}")
                        nc.sync.dma_start(drow, row)
                        sumsT = smallp.tile([P, 8], F32, tag=f"sT{h}")
                        # sumsT[p, s] = sums[s*128 + p]
                        nc.sync.dma_start(
                            sumsT, drow.rearrange("(s p) -> p s", p=P))
                        rT = smallp.tile([P, 8], F32, tag=f"rT{h}")
                        nc.vector.reciprocal(rT, sumsT)
                        recips.append(rT)
                        nc.vector.tensor_copy(mix2[h * 64:(h + 1) * 64, :],
                                              pv_ps[h][0:64, :])
                    for s in range(8):
                        ops = [psp.tile([P, D], F32, tag=f"pv{h}", name=f"op{h}")
                               for h in range(2)]
                        for h in range(2):
                            _mm(nc, ops[h],
                                mix2[h * 64:(h + 1) * 64, s * P:(s + 1) * P],
                                wo_sb[h * 64:(h + 1) * 64, :],
                                start=True, stop=True)
                        t0 = outp.tile([P, D], F32, tag="t0")
                        nc.vector.tensor_scalar_mul(t0, ops[0],
                                                    recips[0][:, s:s + 1])
                        t1 = outp.tile([P, D], F32, tag="t1")
                        nc.vector.tensor_scalar_mul(t1, ops[1],
                                                    recips[1][:, s:s + 1])
                        ob = outp.tile([P, D], F32, tag="ob")
                        nc.gpsimd.tensor_add(ob, t0, t1)
                        nc.sync.dma_start(
                            out[q0 + s * P:q0 + (s + 1) * P, :], ob)

    nc.compile()
    return nc


def get_nc():
    global _NC
    if _NC is None:
        _NC = build()
    return _NC


def make_in_maps(q, k, v, Wq, bq, Wk, bk, Wv, bv, Wo, bo):
    q = np.asarray(q, np.float32)
    k = np.asarray(k, np.float32)
    v = np.asarray(v, np.float32)
    Wq = np.asarray(Wq, np.float32)
    Wk = np.asarray(Wk, np.float32)
    Wv = np.asarray(Wv, np.float32)
    Wo = np.asarray(Wo, np.float32)
    bq = np.asarray(bq, np.float32)
    bk = np.asarray(bk, np.float32)
    bv = np.asarray(bv, np.float32)
    xts = {}
    for n in range(2):
        xts[n] = (np.ascontiguousarray(q[n].T),
                  np.ascontiguousarray(k[n].T),
                  np.ascontiguousarray(v[n].T))
    in_maps = []
    for c in range(8):
        n, hp = c // 4, c % 4
        sl = slice(P * hp, P * (hp + 1))
        xq, xk, xv = xts[n]
        in_maps.append({
            "xqt": xq, "xkt": xk, "xvt": xv,
            "wq": np.ascontiguousarray(Wq[:, sl]),
            "wk": np.ascontiguousarray(Wk[:, sl]),
            "wv": np.ascontiguousarray(Wv[:, sl]),
            "wo": np.ascontiguousarray(Wo[sl, :]),
            "bqs": (bq[sl] * 0.125).reshape(P, 1).astype(np.float32),
            "bkc": bk[sl].reshape(P, 1).astype(np.float32),
            "bvr": bv[sl].reshape(1, P).astype(np.float32),
        })
    return in_maps


def assemble(results, bo):
    bo = np.asarray(bo, np.float32)
    out = np.zeros((2, L, D), np.float32)
    for c in range(8):
        out[c // 4] += results[c]["out"]
    out += bo[None, None, :]
    return out


def kernel(q, k, v, Wq, bq, Wk, bk, Wv, bv, Wo, bo):
    nc = get_nc()
    in_maps = make_in_maps(q, k, v, Wq, bq, Wk, bk, Wv, bv, Wo, bo)
    res = bass_utils.run_bass_kernel_spmd(nc, in_maps, core_ids=list(range(8)))
    return assemble(res.results, bo)


if __name__ == "__main__":
    build()
    print("build ok")



# revision 14
# speedup vs baseline: 1.1053x; 1.1053x over previous
"""Multi-head attention (q/k/v projections + softmax attention + out-projection)
on 8 Trainium2 NeuronCores.

Sharding: 16 (batch, head) units over 8 cores -> core c handles batch n = c//4
and head pair hp = c%4 (columns 128*hp : 128*hp+128 of the projections).
Per-core partial outputs (each pair's contribution to mix @ Wo) are summed on
host per batch, + bo.

Device kernel (per core), v2:
  - Host pre-transposes q[n],k[n],v[n] -> xT [512, 4096] and converts x and
    Wq/Wk/Wv to bf16 (halves input DMA; bf16 rhs streams 1 cy/row on the PE
    at any N, fixing the fp32r N=128 4x penalty on the V-projection).
  - Projections are interleaved into lqc0's attention blocks (chunk b+2
    projected during attention block b), so ScalarE exp starts ~5us in
    instead of idling through a 40us projection phase.
  - Attention in S^T orientation: S^T[lkv,lq] = KPT_h.T @ QPT_h (fp32r,
    K=64), exp on ScalarE [128,1024] tiles, PV accumulates mixT[c,lq] +
    sum(exp) via a ones-column in VP (transpose-free).
  - softmax normalization: sum(exp) psum row is partition-broadcast on
    GPSIMD, then one DVE tensor_tensor divide folds the normalization into
    the PSUM->SBUF move of mix (replaces the DRAM-bounce transpose +
    per-chunk scalar muls + Pool adds of v1).
  - Out-projection is deferred to a tail phase (mix kept in SBUF per lqc):
    the attention j-loop's PSUM tags are never contended, removing the
    per-lqc pipeline stalls. h0+h1 accumulate into one psum tile; output
    DMA'd as bf16 (host sums partials in f32).
  - One unified 8-bank PSUM pool: st0/st1 (S^T tiles + K/Q proj psums) and
    pv0/pv1 (PV accumulators + V proj psums), [128,1024] fp32 = 2 banks each.
"""

import numpy as np
import ml_dtypes

import concourse.bacc as bacc
import concourse.mybir as mybir
import concourse.tile as tile
from concourse import bass_utils

P = 128
L = 4096
D = 512
F32 = mybir.dt.float32
F32R = mybir.dt.float32r
BF16 = mybir.dt.bfloat16
AF = mybir.ActivationFunctionType
ALU = mybir.AluOpType

_NC = None
V_NORM = True   # new broadcast/divide normalization
V_TAIL = True   # deferred out-proj tail
V_ONES = True   # bf16 ones/bias matmul (False: f32r like baseline)
V_INTER = True  # interleave projections into lqc0 attention
V_NOTAIL = False  # skip out-proj tail entirely (debug)
V_TAILN = 32    # number of tail (lqc,s) iterations to emit


def build():
    nc = bacc.Bacc("TRN2", target_bir_lowering=False, debug=False)

    xqt = nc.dram_tensor("xqt", (D, L), BF16, kind="ExternalInput").ap()
    xkt = nc.dram_tensor("xkt", (D, L), BF16, kind="ExternalInput").ap()
    xvt = nc.dram_tensor("xvt", (D, L), BF16, kind="ExternalInput").ap()
    wq = nc.dram_tensor("wq", (D, P), BF16, kind="ExternalInput").ap()
    wk = nc.dram_tensor("wk", (D, P), BF16, kind="ExternalInput").ap()
    wv = nc.dram_tensor("wv", (D, P), BF16, kind="ExternalInput").ap()
    wo = nc.dram_tensor("wo", (P, D), F32R, kind="ExternalInput").ap()
    bqs = nc.dram_tensor("bqs", (P, 1), F32, kind="ExternalInput").ap()
    bkc = nc.dram_tensor("bkc", (P, 1), F32, kind="ExternalInput").ap()
    bvr = nc.dram_tensor("bvr", (1, P), BF16 if V_ONES else F32R,
                         kind="ExternalInput").ap()
    out = nc.dram_tensor("out", (L, D), BF16, kind="ExternalOutput").ap()

    with tile.TileContext(nc) as tc:
        with tc.tile_pool(name="const", bufs=1) as const, \
             tc.tile_pool(name="persist", bufs=1) as persist:
            xqv = xqt.rearrange("(o p) l -> p o l", p=P)
            xkv = xkt.rearrange("(o p) l -> p o l", p=P)
            xvv = xvt.rearrange("(o p) l -> p o l", p=P)

            wk_sb = const.tile([P, 4, P], BF16, tag="wk")
            nc.sync.dma_start(wk_sb, wk.rearrange("(o p) m -> p o m", p=P))
            wq_sb = const.tile([P, 4, P], BF16, tag="wq")
            nc.sync.dma_start(wq_sb, wq.rearrange("(o p) m -> p o m", p=P))
            wv_sb = const.tile([P, 4, P], BF16, tag="wv")
            nc.sync.dma_start(wv_sb, wv.rearrange("(o p) m -> p o m", p=P))
            wo_sb = const.tile([P, D], F32R, tag="wo")
            nc.sync.dma_start(wo_sb, wo)
            bq_sb = const.tile([P, 1], F32, tag="bq")
            nc.sync.dma_start(bq_sb, bqs)
            bk_sb = const.tile([P, 1], F32, tag="bk")
            nc.sync.dma_start(bk_sb, bkc)
            bvr_sb = const.tile([1, P], BF16 if V_ONES else F32R, tag="bvr")
            nc.sync.dma_start(bvr_sb, bvr)
            onesr = const.tile([1, P], BF16 if V_ONES else F32R, tag="onesr")
            nc.scalar.activation(onesr, bvr_sb, AF.Identity,
                                 bias=1.0, scale=0.0)

            qpt_t = [persist.tile([P, 512], F32R, tag=f"qpt{c}",
                                  name=f"qpt{c}") for c in range(8)]
            kpt_t = [persist.tile([P, 512], F32R, tag=f"kpt{c}",
                                  name=f"kpt{c}") for c in range(8)]
            vp_t = [persist.tile([P, 4, 130], F32R, tag=f"vp{c}",
                                 name=f"vp{c}") for c in range(8)]
            mix_t = [persist.tile([P, 1024], F32R, tag=f"mix{l}",
                                  name=f"mix{l}") for l in range(4)]
            ones_in = bq_sb[:, :, None].to_broadcast((P, 4, 1))
            for c in range(8):
                nc.scalar.activation(vp_t[c][:, :, 64:65], ones_in,
                                     AF.Identity, bias=1.0, scale=0.0)
                nc.scalar.activation(vp_t[c][:, :, 129:130], ones_in,
                                     AF.Identity, bias=1.0, scale=0.0)

            with tc.tile_pool(name="xs", bufs=2) as xs, \
                 tc.tile_pool(name="psp", bufs=1, space="PSUM") as psp, \
                 tc.tile_pool(name="esp", bufs=4) as esp, \
                 tc.tile_pool(name="bcp", bufs=2) as bcp, \
                 tc.tile_pool(name="obp", bufs=6) as obp:

                def proj_kv(ch):
                    sl = slice(ch * 512, (ch + 1) * 512)
                    xtk = xs.tile([P, 4, 512], BF16, tag="xtk")
                    nc.sync.dma_start(xtk, xkv[:, :, sl])
                    kps = psp.tile([P, 512], F32, tag="st0", name="kps")
                    for dk in range(4):
                        nc.tensor.matmul(kps, lhsT=wk_sb[:, dk, :],
                                         rhs=xtk[:, dk, :],
                                         start=(dk == 0), stop=(dk == 3))
                    nc.vector.tensor_scalar(kpt_t[ch][:], kps, 1.0, bk_sb,
                                            ALU.mult, ALU.add)
                    xtv = xs.tile([P, 4, 512], BF16, tag="xtv")
                    nc.sync.dma_start(xtv, xvv[:, :, sl])
                    for js in range(4):
                        psv = psp.tile([P, P], F32, tag="st1", name="psv")
                        for dk in range(4):
                            nc.tensor.matmul(psv,
                                             lhsT=xtv[:, dk, js * P:(js + 1) * P],
                                             rhs=wv_sb[:, dk, :],
                                             start=(dk == 0), stop=False)
                        nc.tensor.matmul(psv, lhsT=onesr, rhs=bvr_sb,
                                         start=False, stop=True)
                        nc.vector.tensor_copy(vp_t[ch][:, js, 0:64],
                                              psv[:, 0:64])
                        nc.vector.tensor_copy(vp_t[ch][:, js, 65:129],
                                              psv[:, 64:128])

                def proj_q(ch):
                    sl = slice(ch * 512, (ch + 1) * 512)
                    xtq = xs.tile([P, 4, 512], BF16, tag="xtq")
                    nc.sync.dma_start(xtq, xqv[:, :, sl])
                    qps = psp.tile([P, 512], F32, tag="st0", name="qps")
                    for dk in range(4):
                        nc.tensor.matmul(qps, lhsT=wq_sb[:, dk, :],
                                         rhs=xtq[:, dk, :],
                                         start=(dk == 0), stop=(dk == 3))
                    nc.vector.tensor_scalar(qpt_t[ch][:], qps, 0.125, bq_sb,
                                            ALU.mult, ALU.add)

                # prologue: chunks 0,1 of all projections
                proj_kv(0)
                proj_q(0)
                proj_kv(1)
                proj_q(1)
                if not V_INTER:
                    for ch in range(2, 8):
                        proj_kv(ch)
                        proj_q(ch)

                for lqc in range(4):
                    pv_ps = [psp.tile([P, 1024], F32, tag=f"pv{h}",
                                      name=f"pv{h}") for h in range(2)]
                    for b in range(8):
                        for j in range(4 * b, 4 * b + 4):
                            for h in range(2):
                                hb = h * 64
                                st = psp.tile([P, 1024], F32, tag=f"st{h}")
                                for hf in range(2):
                                    nc.tensor.matmul(
                                        st[:, hf * 512:(hf + 1) * 512],
                                        lhsT=kpt_t[j // 4][hb:hb + 64,
                                                           (j % 4) * P:(j % 4 + 1) * P],
                                        rhs=qpt_t[2 * lqc + hf][hb:hb + 64, :],
                                        start=True, stop=True)
                                est = esp.tile([P, 1024], F32R, tag=f"est{h}")
                                nc.scalar.activation(est, st, AF.Exp)
                                for hf in range(2):
                                    nc.tensor.matmul(
                                        pv_ps[h][0:65, hf * 512:(hf + 1) * 512],
                                        lhsT=vp_t[j // 4][:, j % 4,
                                                          h * 65:(h + 1) * 65],
                                        rhs=est[:, hf * 512:(hf + 1) * 512],
                                        start=(j == 0), stop=(j == 31))
                        # interleaved projections (consumed 2 blocks later /
                        # by later lqc's)
                        if V_INTER and lqc == 0 and b < 6:
                            proj_kv(b + 2)
                        if V_INTER and b in (0, 4) and lqc < 3:
                            qch = 2 * lqc + 2 + (b // 4)
                            proj_q(qch)
                    # normalize: mix = pv[0:64] * broadcast(1/sumexp row)
                    for h in range(2):
                        if V_NORM:
                            row = bcp.tile([1, 1024], F32, tag=f"row{h}")
                            nc.vector.tensor_copy(row, pv_ps[h][64:65, :])
                            rr = bcp.tile([1, 1024], F32, tag=f"rr{h}")
                            nc.vector.reciprocal(rr, row)
                            bc = bcp.tile([P, 1024], F32, tag=f"bc{h}")
                            nc.gpsimd.partition_broadcast(bc, rr)
                            nc.vector.tensor_tensor(
                                mix_t[lqc][h * 64:(h + 1) * 64, :],
                                pv_ps[h][0:64, :], bc[0:64, :], ALU.mult)
                        else:
                            nc.vector.tensor_copy(
                                mix_t[lqc][h * 64:(h + 1) * 64, :],
                                pv_ps[h][0:64, :])

                # tail: out-projection. mix rows 0:64 = h0 c-dims,
                # 64:128 = h1 c-dims, so a single K=128 matmul against
                # wo_sb sums both heads' contributions.
                tags = ["st0", "st1", "pv0", "pv1"]
                for lqc in range(4):
                    q0 = lqc * 1024
                    for s in range(8):
                        i = lqc * 8 + s
                        ops = psp.tile([P, D], F32, tag=tags[i % 4], name="ops")
                        nc.tensor.matmul(
                            ops, lhsT=mix_t[lqc][:, s * P:(s + 1) * P],
                            rhs=wo_sb, start=True, stop=True)
                        ob = obp.tile([P, D], BF16, tag=f"ob{i % 3}")
                        if V_TAIL and i % 2 == 0:
                            nc.scalar.copy(ob, ops)
                        else:
                            nc.vector.tensor_copy(ob, ops)
                        nc.sync.dma_start(
                            out[q0 + s * P:q0 + (s + 1) * P, :], ob)

    nc.compile()
    return nc


def get_nc():
    global _NC
    if _NC is None:
        _NC = build()
    return _NC


def make_in_maps(q, k, v, Wq, bq, Wk, bk, Wv, bv, Wo, bo):
    bf = ml_dtypes.bfloat16
    q = np.asarray(q, np.float32)
    k = np.asarray(k, np.float32)
    v = np.asarray(v, np.float32)
    Wq = np.asarray(Wq, np.float32)
    Wk = np.asarray(Wk, np.float32)
    Wv = np.asarray(Wv, np.float32)
    Wo = np.asarray(Wo, np.float32)
    bq = np.asarray(bq, np.float32)
    bk = np.asarray(bk, np.float32)
    bv = np.asarray(bv, np.float32)
    xts = {}
    for n in range(2):
        xts[n] = (np.ascontiguousarray(q[n].T).astype(bf),
                  np.ascontiguousarray(k[n].T).astype(bf),
                  np.ascontiguousarray(v[n].T).astype(bf))
    in_maps = []
    for c in range(8):
        n, hp = c // 4, c % 4
        sl = slice(P * hp, P * (hp + 1))
        xq, xk, xv = xts[n]
        in_maps.append({
            "xqt": xq, "xkt": xk, "xvt": xv,
            "wq": np.ascontiguousarray(Wq[:, sl]).astype(bf),
            "wk": np.ascontiguousarray(Wk[:, sl]).astype(bf),
            "wv": np.ascontiguousarray(Wv[:, sl]).astype(bf),
            "wo": np.ascontiguousarray(Wo[sl, :]),
            "bqs": (bq[sl] * 0.125).reshape(P, 1).astype(np.float32),
            "bkc": bk[sl].reshape(P, 1).astype(np.float32),
            "bvr": bv[sl].reshape(1, P).astype(bf),
        })
    return in_maps


def assemble(results, bo):
    bo = np.asarray(bo, np.float32)
    out = np.zeros((2, L, D), np.float32)
    for c in range(8):
        out[c // 4] += np.asarray(results[c]["out"], dtype=np.float32)
    out += bo[None, None, :]
    return out


def kernel(q, k, v, Wq, bq, Wk, bk, Wv, bv, Wo, bo):
    nc = get_nc()
    in_maps = make_in_maps(q, k, v, Wq, bq, Wk, bk, Wv, bv, Wo, bo)
    res = bass_utils.run_bass_kernel_spmd(nc, in_maps, core_ids=list(range(8)))
    return assemble(res.results, bo)


if __name__ == "__main__":
    build()
    print("build ok")


# revision 18
# speedup vs baseline: 1.3844x; 1.2525x over previous
"""Multi-head attention (q/k/v projections + softmax attention + out-projection)
on 8 Trainium2 NeuronCores.

Sharding: 16 (batch, head) units over 8 cores -> core c handles batch n = c//4
and head pair hp = c%4 (columns 128*hp : 128*hp+128 of the projections).
Per-core partial outputs (each pair's contribution to mix @ Wo) are summed on
host per batch, + bo.

Device kernel (per core), v6:
  - Host pre-transposes q[n],k[n],v[n] -> xT [512, 4096] and converts x and
    Wq/Wk/Wv to bf16 (halves input DMA; bf16 rhs streams 1 cy/row on the PE
    at any N, fixing the fp32r N=128 4x penalty on the V-projection).
  - Projections are interleaved into the first lq-window's attention blocks,
    so ScalarE exp starts a few us in instead of idling through a serial
    projection phase.
  - Attention runs on 512-wide lq windows (8 of them). Per window the two
    PV accumulators are [128,512] = 1 PSUM bank each, which frees budget
    for THREE [128, 2(kv),512] S^T tiles: the S->exp->free round-trip is
    no longer the pipeline pacer (with 2 buffers it was).
  - S^T in fp32r (K=64); exp on ScalarE over [128, 2x512] tiles (one per
    kv-chunk-pair and head); PV in bf16 accumulates mixT[c,lq] + sum(exp)
    via a ones-column in VP (transpose-free). PV runs one pair behind exp
    (software pipelining) so it never blocks the S/exp chain.
  - ~19% of est tiles (h1, odd pair, later windows) are computed on the DVE
    instead via a Schraudolph fast-exp (bitcast_bf16(int16(x*c1+c2)), one
    tensor_scalar op) - whole-tile offload spreads the approximation across
    kv so softmax dilutes it; this keeps ScalarE below the PE's pace.
  - softmax normalization: 1/sum(exp) from the PSUM row on DVE, GPSIMD
    partition-broadcast, one DVE multiply folds normalization into the
    PSUM->SBUF move of mix.
  - Out-projection is deferred to a tail phase: mix rows are h0|h1 c-dims,
    so one K=128 matmul against Wo sums both heads; output DMA'd as bf16
    (host sums partials in f32).
"""

import numpy as np
import ml_dtypes

import concourse.bacc as bacc
import concourse.mybir as mybir
import concourse.tile as tile
from concourse import bass_utils

P = 128
L = 4096
D = 512
F32 = mybir.dt.float32
F32R = mybir.dt.float32r
BF16 = mybir.dt.bfloat16
I16 = mybir.dt.int16
AF = mybir.ActivationFunctionType
ALU = mybir.AluOpType

_NC = None
V_OFFL_LQC = 2   # DVE fast-exp for h1/odd pairs in lq windows >= this


def build():
    nc = bacc.Bacc("TRN2", target_bir_lowering=False, debug=False)

    xqt = nc.dram_tensor("xqt", (D, L), BF16, kind="ExternalInput").ap()
    xkt = nc.dram_tensor("xkt", (D, L), BF16, kind="ExternalInput").ap()
    xvt = nc.dram_tensor("xvt", (D, L), BF16, kind="ExternalInput").ap()
    wq = nc.dram_tensor("wq", (D, P), BF16, kind="ExternalInput").ap()
    wk = nc.dram_tensor("wk", (D, P), BF16, kind="ExternalInput").ap()
    wv = nc.dram_tensor("wv", (D, P), BF16, kind="ExternalInput").ap()
    wo = nc.dram_tensor("wo", (P, D), F32R, kind="ExternalInput").ap()
    bqs = nc.dram_tensor("bqs", (P, 1), F32, kind="ExternalInput").ap()
    bkc = nc.dram_tensor("bkc", (P, 1), F32, kind="ExternalInput").ap()
    bvr = nc.dram_tensor("bvr", (1, P), BF16, kind="ExternalInput").ap()
    out = nc.dram_tensor("out", (L, D), BF16, kind="ExternalOutput").ap()

    with tile.TileContext(nc) as tc:
        with tc.tile_pool(name="const", bufs=1) as const, \
             tc.tile_pool(name="persist", bufs=1) as persist:
            xqv = xqt.rearrange("(o p) l -> p o l", p=P)
            xkv = xkt.rearrange("(o p) l -> p o l", p=P)
            xvv = xvt.rearrange("(o p) l -> p o l", p=P)

            wk_sb = const.tile([P, 4, P], BF16, tag="wk")
            nc.sync.dma_start(wk_sb, wk.rearrange("(o p) m -> p o m", p=P))
            wq_sb = const.tile([P, 4, P], BF16, tag="wq")
            nc.sync.dma_start(wq_sb, wq.rearrange("(o p) m -> p o m", p=P))
            wv_sb = const.tile([P, 4, P], BF16, tag="wv")
            nc.sync.dma_start(wv_sb, wv.rearrange("(o p) m -> p o m", p=P))
            wo_sb = const.tile([P, D], F32R, tag="wo")
            nc.sync.dma_start(wo_sb, wo)
            bq_sb = const.tile([P, 1], F32, tag="bq")
            nc.sync.dma_start(bq_sb, bqs)
            bk_sb = const.tile([P, 1], F32, tag="bk")
            nc.sync.dma_start(bk_sb, bkc)
            bvr_sb = const.tile([1, P], BF16, tag="bvr")
            nc.sync.dma_start(bvr_sb, bvr)
            onesr = const.tile([1, P], BF16, tag="onesr")
            nc.scalar.activation(onesr, bvr_sb, AF.Identity,
                                 bias=1.0, scale=0.0)

            qpt_t = [persist.tile([P, 512], F32R, tag=f"qpt{c}",
                                  name=f"qpt{c}") for c in range(8)]
            kpt_t = [persist.tile([P, 512], F32R, tag=f"kpt{c}",
                                  name=f"kpt{c}") for c in range(8)]
            vp_t = [persist.tile([P, 4, 130], BF16, tag=f"vp{c}",
                                 name=f"vp{c}") for c in range(8)]
            mix_t = [persist.tile([P, 512], F32R, tag=f"mix{l}",
                                  name=f"mix{l}") for l in range(8)]
            ones_in = bq_sb[:, :, None].to_broadcast((P, 4, 1))
            for c in range(8):
                nc.vector.tensor_scalar(vp_t[c][:, :, 64:65], ones_in,
                                        0.0, 1.0, ALU.mult, ALU.add)
                nc.vector.tensor_scalar(vp_t[c][:, :, 129:130], ones_in,
                                        0.0, 1.0, ALU.mult, ALU.add)

            with tc.tile_pool(name="xs", bufs=2) as xs, \
                 tc.tile_pool(name="psp", bufs=1, space="PSUM") as psp, \
                 tc.tile_pool(name="esp", bufs=4) as esp, \
                 tc.tile_pool(name="bcp", bufs=1) as bcp, \
                 tc.tile_pool(name="obp", bufs=2) as obp:
                stc = [0]  # st tag rotation counter (3 tags x 2 banks)

                def st_tile(shape, name):
                    t = psp.tile(shape, F32, tag=f"st{stc[0] % 3}", name=name)
                    stc[0] += 1
                    return t

                def proj_kv(ch):
                    sl = slice(ch * 512, (ch + 1) * 512)
                    xtk = xs.tile([P, 4, 512], BF16, tag="xtk")
                    nc.sync.dma_start(xtk, xkv[:, :, sl])
                    kps = st_tile([P, 512], "kps")
                    for dk in range(4):
                        nc.tensor.matmul(kps, lhsT=wk_sb[:, dk, :],
                                         rhs=xtk[:, dk, :],
                                         start=(dk == 0), stop=(dk == 3))
                    nc.vector.tensor_scalar(kpt_t[ch][:], kps, 1.0, bk_sb,
                                            ALU.mult, ALU.add)
                    xtv = xs.tile([P, 4, 512], BF16, tag="xtv")
                    nc.sync.dma_start(xtv, xvv[:, :, sl])
                    for js in range(4):
                        psv = st_tile([P, P], "psv")
                        for dk in range(4):
                            nc.tensor.matmul(psv,
                                             lhsT=xtv[:, dk, js * P:(js + 1) * P],
                                             rhs=wv_sb[:, dk, :],
                                             start=(dk == 0), stop=False)
                        nc.tensor.matmul(psv, lhsT=onesr, rhs=bvr_sb,
                                         start=False, stop=True)
                        nc.vector.tensor_copy(vp_t[ch][:, js, 0:64],
                                              psv[:, 0:64])
                        nc.vector.tensor_copy(vp_t[ch][:, js, 65:129],
                                              psv[:, 64:128])

                def proj_q(ch):
                    sl = slice(ch * 512, (ch + 1) * 512)
                    xtq = xs.tile([P, 4, 512], BF16, tag="xtq")
                    nc.sync.dma_start(xtq, xqv[:, :, sl])
                    qps = st_tile([P, 512], "qps")
                    for dk in range(4):
                        nc.tensor.matmul(qps, lhsT=wq_sb[:, dk, :],
                                         rhs=xtq[:, dk, :],
                                         start=(dk == 0), stop=(dk == 3))
                    nc.vector.tensor_scalar(qpt_t[ch][:], qps, 0.125, bq_sb,
                                            ALU.mult, ALU.add)

                # prologue: chunks 0,1 of all projections
                proj_kv(0)
                proj_q(0)
                proj_kv(1)
                proj_q(1)

                # Schraudolph fast-exp in bf16: exp(x) ~=
                # bitcast_bf16(int16(x * 2^7/ln2 + (127*2^7 - 7.42)))
                SC1, SC2 = 184.6650292, 16249.0

                def emit_pv(pair, ests, pv_ps):
                    for h in range(2):
                        for r in range(2):
                            j = 2 * pair + r
                            nc.tensor.matmul(
                                pv_ps[h][0:65, :],
                                lhsT=vp_t[j // 4][:, j % 4,
                                                  h * 65:(h + 1) * 65],
                                rhs=ests[h][:, r, :],
                                start=(j == 0), stop=(j == 31))

                for lqc in range(8):
                    pv_ps = [psp.tile([P, 512], F32, tag=f"pv{h}",
                                      name=f"pv{h}") for h in range(2)]
                    prev = None
                    for b in range(8):
                        for pair in range(2 * b, 2 * b + 2):
                            ests = []
                            for h in range(2):
                                hb = h * 64
                                st = st_tile([P, 2, 512], "st")
                                for r in range(2):
                                    j = 2 * pair + r
                                    nc.tensor.matmul(
                                        st[:, r, :],
                                        lhsT=kpt_t[j // 4][hb:hb + 64,
                                                           (j % 4) * P:(j % 4 + 1) * P],
                                        rhs=qpt_t[lqc][hb:hb + 64, :],
                                        start=True, stop=True)
                                est = esp.tile([P, 2, 512], BF16,
                                               tag=f"est{h}")
                                if (lqc >= V_OFFL_LQC and h == 1
                                        and pair % 8 in (1, 3, 5)):
                                    nc.vector.tensor_scalar(
                                        est[:, :, :].bitcast(I16),
                                        st, SC1, SC2, ALU.mult, ALU.add)
                                else:
                                    nc.scalar.activation(est, st, AF.Exp)
                                ests.append(est)
                            if prev is not None:
                                emit_pv(prev[0], prev[1], pv_ps)
                            prev = (pair, ests)
                        # interleaved projections (consumed later)
                        if lqc == 0 and b < 6:
                            proj_kv(b + 2)
                        if lqc < 3 and b in (0, 4):
                            qch = 2 * lqc + 2 + (b // 4)
                            if qch < 8:
                                proj_q(qch)
                    emit_pv(prev[0], prev[1], pv_ps)
                    # normalize: mix = pv[0:64] * broadcast(1/sumexp row)
                    for h in range(2):
                        rr = bcp.tile([1, 512], F32, tag=f"rr{h}")
                        nc.vector.reciprocal(rr, pv_ps[h][64:65, :])
                        bc = bcp.tile([P, 512], F32, tag=f"bc{h}")
                        nc.gpsimd.partition_broadcast(bc, rr)
                        nc.vector.tensor_tensor(
                            mix_t[lqc][h * 64:(h + 1) * 64, :],
                            pv_ps[h][0:64, :], bc[0:64, :], ALU.mult)

                # tail: out-projection. mix rows 0:64 = h0 c-dims,
                # 64:128 = h1 c-dims, so a single K=128 matmul against
                # wo_sb sums both heads' contributions.
                tags = ["st0", "st1", "st2", "pv0", "pv1"]
                for lqc in range(8):
                    for s in range(4):
                        i = lqc * 4 + s
                        ops = psp.tile([P, D], F32, tag=tags[i % 5],
                                       name="ops")
                        nc.tensor.matmul(
                            ops, lhsT=mix_t[lqc][:, s * P:(s + 1) * P],
                            rhs=wo_sb, start=True, stop=True)
                        ob = obp.tile([P, D], BF16, tag=f"ob{i % 3}")
                        if i % 2 == 0:
                            nc.scalar.copy(ob, ops)
                        else:
                            nc.vector.tensor_copy(ob, ops)
                        nc.sync.dma_start(
                            out[lqc * 512 + s * P:lqc * 512 + (s + 1) * P, :],
                            ob)

    nc.compile()
    return nc


def get_nc():
    global _NC
    if _NC is None:
        _NC = build()
    return _NC


def make_in_maps(q, k, v, Wq, bq, Wk, bk, Wv, bv, Wo, bo):
    bf = ml_dtypes.bfloat16
    q = np.asarray(q, np.float32)
    k = np.asarray(k, np.float32)
    v = np.asarray(v, np.float32)
    Wq = np.asarray(Wq, np.float32)
    Wk = np.asarray(Wk, np.float32)
    Wv = np.asarray(Wv, np.float32)
    Wo = np.asarray(Wo, np.float32)
    bq = np.asarray(bq, np.float32)
    bk = np.asarray(bk, np.float32)
    bv = np.asarray(bv, np.float32)
    xts = {}
    for n in range(2):
        xts[n] = (np.ascontiguousarray(q[n].T).astype(bf),
                  np.ascontiguousarray(k[n].T).astype(bf),
                  np.ascontiguousarray(v[n].T).astype(bf))
    in_maps = []
    for c in range(8):
        n, hp = c // 4, c % 4
        sl = slice(P * hp, P * (hp + 1))
        xq, xk, xv = xts[n]
        in_maps.append({
            "xqt": xq, "xkt": xk, "xvt": xv,
            "wq": np.ascontiguousarray(Wq[:, sl]).astype(bf),
            "wk": np.ascontiguousarray(Wk[:, sl]).astype(bf),
            "wv": np.ascontiguousarray(Wv[:, sl]).astype(bf),
            "wo": np.ascontiguousarray(Wo[sl, :]),
            "bqs": (bq[sl] * 0.125).reshape(P, 1).astype(np.float32),
            "bkc": bk[sl].reshape(P, 1).astype(np.float32),
            "bvr": bv[sl].reshape(1, P).astype(bf),
        })
    return in_maps


def assemble(results, bo):
    bo = np.asarray(bo, np.float32)
    out = np.zeros((2, L, D), np.float32)
    for c in range(8):
        out[c // 4] += np.asarray(results[c]["out"], dtype=np.float32)
    out += bo[None, None, :]
    return out


def kernel(q, k, v, Wq, bq, Wk, bk, Wv, bv, Wo, bo):
    nc = get_nc()
    in_maps = make_in_maps(q, k, v, Wq, bq, Wk, bk, Wv, bv, Wo, bo)
    res = bass_utils.run_bass_kernel_spmd(nc, in_maps, core_ids=list(range(8)))
    return assemble(res.results, bo)


if __name__ == "__main__":
    build()
    print("build ok")


# revision 19
# speedup vs baseline: 1.3885x; 1.0030x over previous
"""Multi-head attention (q/k/v projections + softmax attention + out-projection)
on 8 Trainium2 NeuronCores.

Sharding: 16 (batch, head) units over 8 cores -> core c handles batch n = c//4
and head pair hp = c%4 (columns 128*hp : 128*hp+128 of the projections).
Per-core partial outputs (each pair's contribution to mix @ Wo) are summed on
host per batch, + bo.

Device kernel (per core), v6:
  - Host pre-transposes q[n],k[n],v[n] -> xT [512, 4096] and converts x and
    Wq/Wk/Wv to bf16 (halves input DMA; bf16 rhs streams 1 cy/row on the PE
    at any N, fixing the fp32r N=128 4x penalty on the V-projection).
  - Projections are interleaved into the first lq-window's attention blocks,
    so ScalarE exp starts a few us in instead of idling through a serial
    projection phase.
  - Attention runs on 512-wide lq windows (8 of them). Per window the two
    PV accumulators are [128,512] = 1 PSUM bank each, which frees budget
    for THREE [128, 2(kv),512] S^T tiles: the S->exp->free round-trip is
    no longer the pipeline pacer (with 2 buffers it was).
  - S^T in fp32r (K=64); exp on ScalarE over [128, 2x512] tiles (one per
    kv-chunk-pair and head); PV in bf16 accumulates mixT[c,lq] + sum(exp)
    via a ones-column in VP (transpose-free). PV runs one pair behind exp
    (software pipelining) so it never blocks the S/exp chain.
  - ~19% of est tiles (h1, odd pair, later windows) are computed on the DVE
    instead via a Schraudolph fast-exp (bitcast_bf16(int16(x*c1+c2)), one
    tensor_scalar op) - whole-tile offload spreads the approximation across
    kv so softmax dilutes it; this keeps ScalarE below the PE's pace.
  - softmax normalization: 1/sum(exp) from the PSUM row on DVE, GPSIMD
    partition-broadcast, one DVE multiply folds normalization into the
    PSUM->SBUF move of mix.
  - Out-projection is deferred to a tail phase: mix rows are h0|h1 c-dims,
    so one K=128 matmul against Wo sums both heads; output DMA'd as bf16
    (host sums partials in f32).
"""

import numpy as np
import ml_dtypes

import concourse.bacc as bacc
import concourse.mybir as mybir
import concourse.tile as tile
from concourse import bass_utils

P = 128
L = 4096
D = 512
F32 = mybir.dt.float32
F32R = mybir.dt.float32r
BF16 = mybir.dt.bfloat16
I16 = mybir.dt.int16
AF = mybir.ActivationFunctionType
ALU = mybir.AluOpType

_NC = None
V_OFFL_LQC = 2   # DVE fast-exp for h1/odd pairs in lq windows >= this


def build():
    nc = bacc.Bacc("TRN2", target_bir_lowering=False, debug=False)

    xqt = nc.dram_tensor("xqt", (D, L), BF16, kind="ExternalInput").ap()
    xkt = nc.dram_tensor("xkt", (D, L), BF16, kind="ExternalInput").ap()
    xvt = nc.dram_tensor("xvt", (D, L), BF16, kind="ExternalInput").ap()
    wq = nc.dram_tensor("wq", (D, P), BF16, kind="ExternalInput").ap()
    wk = nc.dram_tensor("wk", (D, P), BF16, kind="ExternalInput").ap()
    wv = nc.dram_tensor("wv", (D, P), BF16, kind="ExternalInput").ap()
    wo = nc.dram_tensor("wo", (P, D), F32R, kind="ExternalInput").ap()
    bqs = nc.dram_tensor("bqs", (P, 1), F32, kind="ExternalInput").ap()
    bkc = nc.dram_tensor("bkc", (P, 1), F32, kind="ExternalInput").ap()
    bvr = nc.dram_tensor("bvr", (1, P), BF16, kind="ExternalInput").ap()
    out = nc.dram_tensor("out", (L, D), BF16, kind="ExternalOutput").ap()

    with tile.TileContext(nc) as tc:
        with tc.tile_pool(name="const", bufs=1) as const, \
             tc.tile_pool(name="persist", bufs=1) as persist:
            xqv = xqt.rearrange("(o p) l -> p o l", p=P)
            xkv = xkt.rearrange("(o p) l -> p o l", p=P)
            xvv = xvt.rearrange("(o p) l -> p o l", p=P)

            wk_sb = const.tile([P, 4, P], BF16, tag="wk")
            nc.sync.dma_start(wk_sb, wk.rearrange("(o p) m -> p o m", p=P))
            wq_sb = const.tile([P, 4, P], BF16, tag="wq")
            nc.sync.dma_start(wq_sb, wq.rearrange("(o p) m -> p o m", p=P))
            wv_sb = const.tile([P, 4, P], BF16, tag="wv")
            nc.sync.dma_start(wv_sb, wv.rearrange("(o p) m -> p o m", p=P))
            bq_sb = const.tile([P, 1], F32, tag="bq")
            nc.sync.dma_start(bq_sb, bqs)
            bk_sb = const.tile([P, 1], F32, tag="bk")
            nc.sync.dma_start(bk_sb, bkc)
            bvr_sb = const.tile([1, P], BF16, tag="bvr")
            nc.sync.dma_start(bvr_sb, bvr)
            onesr = const.tile([1, P], BF16, tag="onesr")
            nc.scalar.activation(onesr, bvr_sb, AF.Identity,
                                 bias=1.0, scale=0.0)

            qpt_t = [persist.tile([P, 512], F32R, tag=f"qpt{c}",
                                  name=f"qpt{c}") for c in range(8)]
            kpt_t = [persist.tile([P, 512], F32R, tag=f"kpt{c}",
                                  name=f"kpt{c}") for c in range(8)]
            vp_t = [persist.tile([P, 4, 130], BF16, tag=f"vp{c}",
                                 name=f"vp{c}") for c in range(8)]
            mix_t = [persist.tile([P, 512], F32R, tag=f"mix{l}",
                                  name=f"mix{l}") for l in range(8)]
            ones_in = bq_sb[:, :, None].to_broadcast((P, 4, 1))
            for c in range(8):
                nc.vector.tensor_scalar(vp_t[c][:, :, 64:65], ones_in,
                                        0.0, 1.0, ALU.mult, ALU.add)
                nc.vector.tensor_scalar(vp_t[c][:, :, 129:130], ones_in,
                                        0.0, 1.0, ALU.mult, ALU.add)

            with tc.tile_pool(name="xs", bufs=2) as xs, \
                 tc.tile_pool(name="psp", bufs=1, space="PSUM") as psp, \
                 tc.tile_pool(name="esp", bufs=4) as esp, \
                 tc.tile_pool(name="bcp", bufs=1) as bcp, \
                 tc.tile_pool(name="obp", bufs=2) as obp:
                stc = [0]  # st tag rotation counter (3 tags x 2 banks)

                def st_tile(shape, name):
                    t = psp.tile(shape, F32, tag=f"st{stc[0] % 3}", name=name)
                    stc[0] += 1
                    return t

                def proj_kv(ch):
                    sl = slice(ch * 512, (ch + 1) * 512)
                    xtk = xs.tile([P, 4, 512], BF16, tag="xtk")
                    nc.sync.dma_start(xtk, xkv[:, :, sl])
                    kps = st_tile([P, 512], "kps")
                    for dk in range(4):
                        nc.tensor.matmul(kps, lhsT=wk_sb[:, dk, :],
                                         rhs=xtk[:, dk, :],
                                         start=(dk == 0), stop=(dk == 3))
                    nc.vector.tensor_scalar(kpt_t[ch][:], kps, 1.0, bk_sb,
                                            ALU.mult, ALU.add)
                    xtv = xs.tile([P, 4, 512], BF16, tag="xtv")
                    nc.sync.dma_start(xtv, xvv[:, :, sl])
                    for js in range(4):
                        psv = st_tile([P, P], "psv")
                        for dk in range(4):
                            nc.tensor.matmul(psv,
                                             lhsT=xtv[:, dk, js * P:(js + 1) * P],
                                             rhs=wv_sb[:, dk, :],
                                             start=(dk == 0), stop=False)
                        nc.tensor.matmul(psv, lhsT=onesr, rhs=bvr_sb,
                                         start=False, stop=True)
                        nc.vector.tensor_copy(vp_t[ch][:, js, 0:64],
                                              psv[:, 0:64])
                        nc.vector.tensor_copy(vp_t[ch][:, js, 65:129],
                                              psv[:, 64:128])

                def proj_q(ch):
                    sl = slice(ch * 512, (ch + 1) * 512)
                    xtq = xs.tile([P, 4, 512], BF16, tag="xtq")
                    nc.sync.dma_start(xtq, xqv[:, :, sl])
                    qps = st_tile([P, 512], "qps")
                    for dk in range(4):
                        nc.tensor.matmul(qps, lhsT=wq_sb[:, dk, :],
                                         rhs=xtq[:, dk, :],
                                         start=(dk == 0), stop=(dk == 3))
                    nc.vector.tensor_scalar(qpt_t[ch][:], qps, 0.125, bq_sb,
                                            ALU.mult, ALU.add)

                # prologue: chunks 0,1 of all projections
                proj_kv(0)
                proj_q(0)
                proj_kv(1)
                proj_q(1)
                wo_sb = const.tile([P, D], F32R, tag="wo")
                nc.sync.dma_start(wo_sb, wo)

                # Schraudolph fast-exp in bf16: exp(x) ~=
                # bitcast_bf16(int16(x * 2^7/ln2 + (127*2^7 - 7.42)))
                SC1, SC2 = 184.6650292, 16249.0

                def emit_pv(pair, ests, pv_ps):
                    for h in range(2):
                        for r in range(2):
                            j = 2 * pair + r
                            nc.tensor.matmul(
                                pv_ps[h][0:65, :],
                                lhsT=vp_t[j // 4][:, j % 4,
                                                  h * 65:(h + 1) * 65],
                                rhs=ests[h][:, r, :],
                                start=(j == 0), stop=(j == 31))

                for lqc in range(8):
                    pv_ps = [psp.tile([P, 512], F32, tag=f"pv{h}",
                                      name=f"pv{h}") for h in range(2)]
                    prev = None
                    for b in range(8):
                        for pair in range(2 * b, 2 * b + 2):
                            ests = []
                            for h in range(2):
                                hb = h * 64
                                st = st_tile([P, 2, 512], "st")
                                for r in range(2):
                                    j = 2 * pair + r
                                    nc.tensor.matmul(
                                        st[:, r, :],
                                        lhsT=kpt_t[j // 4][hb:hb + 64,
                                                           (j % 4) * P:(j % 4 + 1) * P],
                                        rhs=qpt_t[lqc][hb:hb + 64, :],
                                        start=True, stop=True)
                                est = esp.tile([P, 2, 512], BF16,
                                               tag=f"est{h}")
                                if (lqc >= V_OFFL_LQC and h == 1
                                        and pair % 8 in (1, 3, 5)):
                                    nc.vector.tensor_scalar(
                                        est[:, :, :].bitcast(I16),
                                        st, SC1, SC2, ALU.mult, ALU.add)
                                else:
                                    nc.scalar.activation(est, st, AF.Exp)
                                ests.append(est)
                            if prev is not None:
                                emit_pv(prev[0], prev[1], pv_ps)
                            prev = (pair, ests)
                        # interleaved projections (consumed later)
                        if lqc == 0 and b < 6:
                            proj_kv(b + 2)
                        if lqc < 3 and b in (0, 4):
                            qch = 2 * lqc + 2 + (b // 4)
                            if qch < 8:
                                proj_q(qch)
                    emit_pv(prev[0], prev[1], pv_ps)
                    # free pv banks fast: raw copy to SBUF; normalization
                    # (recip/broadcast/mult into mix) happens lazily and
                    # overlaps the next window's attention.
                    for h in range(2):
                        raw = bcp.tile([P, 512], F32, tag=f"raw{h}", bufs=2)
                        nc.vector.tensor_copy(raw[0:65, :], pv_ps[h][0:65, :])
                        rr = bcp.tile([1, 512], F32, tag=f"rr{h}", bufs=2)
                        nc.vector.reciprocal(rr, raw[64:65, :])
                        bc = bcp.tile([P, 512], F32, tag=f"bc{h}", bufs=2)
                        nc.gpsimd.partition_broadcast(bc, rr)
                        nc.vector.tensor_tensor(
                            mix_t[lqc][h * 64:(h + 1) * 64, :],
                            raw[0:64, :], bc[0:64, :], ALU.mult)

                # tail: out-projection. mix rows 0:64 = h0 c-dims,
                # 64:128 = h1 c-dims, so a single K=128 matmul against
                # wo_sb sums both heads' contributions.
                tags = ["st0", "st1", "st2", "pv0", "pv1"]
                for lqc in range(8):
                    for s in range(4):
                        i = lqc * 4 + s
                        ops = psp.tile([P, D], F32, tag=tags[i % 5],
                                       name="ops")
                        nc.tensor.matmul(
                            ops, lhsT=mix_t[lqc][:, s * P:(s + 1) * P],
                            rhs=wo_sb, start=True, stop=True)
                        ob = obp.tile([P, D], BF16, tag=f"ob{i % 3}")
                        if i % 2 == 0:
                            nc.scalar.copy(ob, ops)
                        else:
                            nc.vector.tensor_copy(ob, ops)
                        nc.sync.dma_start(
                            out[lqc * 512 + s * P:lqc * 512 + (s + 1) * P, :],
                            ob)

    nc.compile()
    return nc


def get_nc():
    global _NC
    if _NC is None:
        _NC = build()
    return _NC


def make_in_maps(q, k, v, Wq, bq, Wk, bk, Wv, bv, Wo, bo):
    bf = ml_dtypes.bfloat16
    q = np.asarray(q, np.float32)
    k = np.asarray(k, np.float32)
    v = np.asarray(v, np.float32)
    Wq = np.asarray(Wq, np.float32)
    Wk = np.asarray(Wk, np.float32)
    Wv = np.asarray(Wv, np.float32)
    Wo = np.asarray(Wo, np.float32)
    bq = np.asarray(bq, np.float32)
    bk = np.asarray(bk, np.float32)
    bv = np.asarray(bv, np.float32)
    xts = {}
    for n in range(2):
        xts[n] = (np.ascontiguousarray(q[n].T).astype(bf),
                  np.ascontiguousarray(k[n].T).astype(bf),
                  np.ascontiguousarray(v[n].T).astype(bf))
    in_maps = []
    for c in range(8):
        n, hp = c // 4, c % 4
        sl = slice(P * hp, P * (hp + 1))
        xq, xk, xv = xts[n]
        in_maps.append({
            "xqt": xq, "xkt": xk, "xvt": xv,
            "wq": np.ascontiguousarray(Wq[:, sl]).astype(bf),
            "wk": np.ascontiguousarray(Wk[:, sl]).astype(bf),
            "wv": np.ascontiguousarray(Wv[:, sl]).astype(bf),
            "wo": np.ascontiguousarray(Wo[sl, :]),
            "bqs": (bq[sl] * 0.125).reshape(P, 1).astype(np.float32),
            "bkc": bk[sl].reshape(P, 1).astype(np.float32),
            "bvr": bv[sl].reshape(1, P).astype(bf),
        })
    return in_maps


def assemble(results, bo):
    bo = np.asarray(bo, np.float32)
    out = np.zeros((2, L, D), np.float32)
    for c in range(8):
        out[c // 4] += np.asarray(results[c]["out"], dtype=np.float32)
    out += bo[None, None, :]
    return out


def kernel(q, k, v, Wq, bq, Wk, bk, Wv, bv, Wo, bo):
    nc = get_nc()
    in_maps = make_in_maps(q, k, v, Wq, bq, Wk, bk, Wv, bv, Wo, bo)
    res = bass_utils.run_bass_kernel_spmd(nc, in_maps, core_ids=list(range(8)))
    return assemble(res.results, bo)


if __name__ == "__main__":
    build()
    print("build ok")


# revision 20
# speedup vs baseline: 1.4018x; 1.0095x over previous
"""Multi-head attention (q/k/v projections + softmax attention + out-projection)
on 8 Trainium2 NeuronCores.

Sharding: 16 (batch, head) units over 8 cores -> core c handles batch n = c//4
and head pair hp = c%4 (columns 128*hp : 128*hp+128 of the projections).
Per-core partial outputs (each pair's contribution to mix @ Wo) are summed on
host per batch, + bo.

Device kernel (per core), v6:
  - Host pre-transposes q[n],k[n],v[n] -> xT [512, 4096] and converts x and
    Wq/Wk/Wv to bf16 (halves input DMA; bf16 rhs streams 1 cy/row on the PE
    at any N, fixing the fp32r N=128 4x penalty on the V-projection).
  - Projections are interleaved into the first lq-window's attention blocks,
    so ScalarE exp starts a few us in instead of idling through a serial
    projection phase.
  - Attention runs on 512-wide lq windows (8 of them). Per window the two
    PV accumulators are [128,512] = 1 PSUM bank each, which frees budget
    for THREE [128, 2(kv),512] S^T tiles: the S->exp->free round-trip is
    no longer the pipeline pacer (with 2 buffers it was).
  - S^T in fp32r (K=64); exp on ScalarE over [128, 2x512] tiles (one per
    kv-chunk-pair and head); PV in bf16 accumulates mixT[c,lq] + sum(exp)
    via a ones-column in VP (transpose-free). PV runs one pair behind exp
    (software pipelining) so it never blocks the S/exp chain.
  - ~19% of est tiles (h1, odd pair, later windows) are computed on the DVE
    instead via a Schraudolph fast-exp (bitcast_bf16(int16(x*c1+c2)), one
    tensor_scalar op) - whole-tile offload spreads the approximation across
    kv so softmax dilutes it; this keeps ScalarE below the PE's pace.
  - softmax normalization: 1/sum(exp) from the PSUM row on DVE, GPSIMD
    partition-broadcast, one DVE multiply folds normalization into the
    PSUM->SBUF move of mix.
  - Out-projection is deferred to a tail phase: mix rows are h0|h1 c-dims,
    so one K=128 matmul against Wo sums both heads; output DMA'd as bf16
    (host sums partials in f32).
"""

import numpy as np
import ml_dtypes

import concourse.bacc as bacc
import concourse.mybir as mybir
import concourse.tile as tile
from concourse import bass_utils

P = 128
L = 4096
D = 512
F32 = mybir.dt.float32
F32R = mybir.dt.float32r
BF16 = mybir.dt.bfloat16
I16 = mybir.dt.int16
AF = mybir.ActivationFunctionType
ALU = mybir.AluOpType

_NC = None
V_OFFL_LQC = 2   # DVE fast-exp for h1/odd pairs in lq windows >= this


def build():
    nc = bacc.Bacc("TRN2", target_bir_lowering=False, debug=False)

    xqt = nc.dram_tensor("xqt", (D, L), BF16, kind="ExternalInput").ap()
    xkt = nc.dram_tensor("xkt", (D, L), BF16, kind="ExternalInput").ap()
    xvt = nc.dram_tensor("xvt", (D, L), BF16, kind="ExternalInput").ap()
    wq = nc.dram_tensor("wq", (D, P), BF16, kind="ExternalInput").ap()
    wk = nc.dram_tensor("wk", (D, P), BF16, kind="ExternalInput").ap()
    wv = nc.dram_tensor("wv", (D, P), BF16, kind="ExternalInput").ap()
    wo = nc.dram_tensor("wo", (P, D), F32R, kind="ExternalInput").ap()
    bqs = nc.dram_tensor("bqs", (P, 1), F32, kind="ExternalInput").ap()
    bkc = nc.dram_tensor("bkc", (P, 1), F32, kind="ExternalInput").ap()
    bvr = nc.dram_tensor("bvr", (1, P), BF16, kind="ExternalInput").ap()
    out = nc.dram_tensor("out", (L, D), BF16, kind="ExternalOutput").ap()

    with tile.TileContext(nc) as tc:
        with tc.tile_pool(name="const", bufs=1) as const, \
             tc.tile_pool(name="persist", bufs=1) as persist:
            xqv = xqt.rearrange("(o p) l -> p o l", p=P)
            xkv = xkt.rearrange("(o p) l -> p o l", p=P)
            xvv = xvt.rearrange("(o p) l -> p o l", p=P)

            wk_sb = const.tile([P, 4, P], BF16, tag="wk")
            nc.sync.dma_start(wk_sb, wk.rearrange("(o p) m -> p o m", p=P))
            wq_sb = const.tile([P, 4, P], BF16, tag="wq")
            nc.sync.dma_start(wq_sb, wq.rearrange("(o p) m -> p o m", p=P))
            wv_sb = const.tile([P, 4, P], BF16, tag="wv")
            nc.sync.dma_start(wv_sb, wv.rearrange("(o p) m -> p o m", p=P))
            bq_sb = const.tile([P, 1], F32, tag="bq")
            nc.sync.dma_start(bq_sb, bqs)
            bk_sb = const.tile([P, 1], F32, tag="bk")
            nc.sync.dma_start(bk_sb, bkc)
            bvr_sb = const.tile([1, P], BF16, tag="bvr")
            nc.sync.dma_start(bvr_sb, bvr)
            onesr = const.tile([1, P], BF16, tag="onesr")
            nc.scalar.activation(onesr, bvr_sb, AF.Identity,
                                 bias=1.0, scale=0.0)

            qpt_t = [persist.tile([P, 512], F32R, tag=f"qpt{c}",
                                  name=f"qpt{c}") for c in range(8)]
            kpt_t = [persist.tile([P, 512], F32R, tag=f"kpt{c}",
                                  name=f"kpt{c}") for c in range(8)]
            vp_t = [persist.tile([P, 4, 130], BF16, tag=f"vp{c}",
                                 name=f"vp{c}") for c in range(8)]
            mix_t = [persist.tile([P, 512], F32R, tag=f"mix{l}",
                                  name=f"mix{l}") for l in range(8)]
            ones_in = bq_sb[:, :, None].to_broadcast((P, 4, 1))
            for c in range(8):
                nc.vector.tensor_scalar(vp_t[c][:, :, 64:65], ones_in,
                                        0.0, 1.0, ALU.mult, ALU.add)
                nc.vector.tensor_scalar(vp_t[c][:, :, 129:130], ones_in,
                                        0.0, 1.0, ALU.mult, ALU.add)

            with tc.tile_pool(name="xs", bufs=2) as xs, \
                 tc.tile_pool(name="psp", bufs=1, space="PSUM") as psp, \
                 tc.tile_pool(name="esp", bufs=4) as esp, \
                 tc.tile_pool(name="bcp", bufs=1) as bcp, \
                 tc.tile_pool(name="obp", bufs=2) as obp:
                stc = [0]  # st tag rotation counter (3 tags x 2 banks)

                def st_tile(shape, name):
                    t = psp.tile(shape, F32, tag=f"st{stc[0] % 3}", name=name)
                    stc[0] += 1
                    return t

                def proj_k(ch):
                    sl = slice(ch * 512, (ch + 1) * 512)
                    xtk = xs.tile([P, 4, 512], BF16, tag="xtk")
                    nc.sync.dma_start(xtk, xkv[:, :, sl])
                    kps = st_tile([P, 512], "kps")
                    for dk in range(4):
                        nc.tensor.matmul(kps, lhsT=wk_sb[:, dk, :],
                                         rhs=xtk[:, dk, :],
                                         start=(dk == 0), stop=(dk == 3))
                    nc.vector.tensor_scalar(kpt_t[ch][:], kps, 1.0, bk_sb,
                                            ALU.mult, ALU.add)

                def proj_v(ch):
                    sl = slice(ch * 512, (ch + 1) * 512)
                    xtv = xs.tile([P, 4, 512], BF16, tag="xtv")
                    nc.gpsimd.dma_start(xtv, xvv[:, :, sl])
                    for js in range(4):
                        psv = st_tile([P, P], "psv")
                        for dk in range(4):
                            nc.tensor.matmul(psv,
                                             lhsT=xtv[:, dk, js * P:(js + 1) * P],
                                             rhs=wv_sb[:, dk, :],
                                             start=(dk == 0), stop=False)
                        nc.tensor.matmul(psv, lhsT=onesr, rhs=bvr_sb,
                                         start=False, stop=True)
                        nc.vector.tensor_copy(vp_t[ch][:, js, 0:64],
                                              psv[:, 0:64])
                        nc.vector.tensor_copy(vp_t[ch][:, js, 65:129],
                                              psv[:, 64:128])

                def proj_q(ch):
                    sl = slice(ch * 512, (ch + 1) * 512)
                    xtq = xs.tile([P, 4, 512], BF16, tag="xtq")
                    nc.gpsimd.dma_start(xtq, xqv[:, :, sl])
                    qps = st_tile([P, 512], "qps")
                    for dk in range(4):
                        nc.tensor.matmul(qps, lhsT=wq_sb[:, dk, :],
                                         rhs=xtq[:, dk, :],
                                         start=(dk == 0), stop=(dk == 3))
                    nc.vector.tensor_scalar(qpt_t[ch][:], qps, 0.125, bq_sb,
                                            ALU.mult, ALU.add)

                def proj_kv(ch):
                    proj_k(ch)
                    proj_v(ch)

                # prologue: chunks 0,1 of all projections
                proj_kv(0)
                proj_q(0)
                proj_kv(1)
                proj_q(1)
                wo_sb = const.tile([P, D], F32R, tag="wo")
                nc.sync.dma_start(wo_sb, wo)

                # Schraudolph fast-exp in bf16: exp(x) ~=
                # bitcast_bf16(int16(x * 2^7/ln2 + (127*2^7 - 7.42)))
                SC1, SC2 = 184.6650292, 16249.0

                def emit_pv(pair, ests, pv_ps):
                    for h in range(2):
                        for r in range(2):
                            j = 2 * pair + r
                            nc.tensor.matmul(
                                pv_ps[h][0:65, :],
                                lhsT=vp_t[j // 4][:, j % 4,
                                                  h * 65:(h + 1) * 65],
                                rhs=ests[h][:, r, :],
                                start=(j == 0), stop=(j == 31))

                for lqc in range(8):
                    pv_ps = [psp.tile([P, 512], F32, tag=f"pv{h}",
                                      name=f"pv{h}") for h in range(2)]
                    prev = None
                    for b in range(8):
                        for pair in range(2 * b, 2 * b + 2):
                            ests = []
                            for h in range(2):
                                hb = h * 64
                                st = st_tile([P, 2, 512], "st")
                                for r in range(2):
                                    j = 2 * pair + r
                                    nc.tensor.matmul(
                                        st[:, r, :],
                                        lhsT=kpt_t[j // 4][hb:hb + 64,
                                                           (j % 4) * P:(j % 4 + 1) * P],
                                        rhs=qpt_t[lqc][hb:hb + 64, :],
                                        start=True, stop=True)
                                est = esp.tile([P, 2, 512], BF16,
                                               tag=f"est{h}")
                                if (lqc >= V_OFFL_LQC and h == 1
                                        and pair % 8 in (1, 3, 5)):
                                    nc.vector.tensor_scalar(
                                        est[:, :, :].bitcast(I16),
                                        st, SC1, SC2, ALU.mult, ALU.add)
                                else:
                                    nc.scalar.activation(est, st, AF.Exp)
                                ests.append(est)
                            if prev is not None:
                                emit_pv(prev[0], prev[1], pv_ps)
                            prev = (pair, ests)
                            # interleaved projections, half-chunk granular
                            if lqc == 0 and b < 6:
                                if pair % 2 == 0:
                                    proj_k(b + 2)
                                else:
                                    proj_v(b + 2)
                        if lqc == 0 and b in (6, 7):
                            proj_q(b - 4)
                        if lqc in (1, 2) and b in (0, 4):
                            qch = 2 * lqc + 2 + (b // 4)
                            if qch < 8:
                                proj_q(qch)
                    emit_pv(prev[0], prev[1], pv_ps)
                    # free pv banks fast: raw copy to SBUF; normalization
                    # (recip/broadcast/mult into mix) happens lazily and
                    # overlaps the next window's attention.
                    for h in range(2):
                        raw = bcp.tile([P, 512], F32, tag=f"raw{h}", bufs=2)
                        nc.vector.tensor_copy(raw[0:65, :], pv_ps[h][0:65, :])
                        rr = bcp.tile([1, 512], F32, tag=f"rr{h}", bufs=2)
                        nc.vector.reciprocal(rr, raw[64:65, :])
                        bc = bcp.tile([P, 512], F32, tag=f"bc{h}", bufs=2)
                        nc.gpsimd.partition_broadcast(bc, rr)
                        nc.vector.tensor_tensor(
                            mix_t[lqc][h * 64:(h + 1) * 64, :],
                            raw[0:64, :], bc[0:64, :], ALU.mult)

                # tail: out-projection. mix rows 0:64 = h0 c-dims,
                # 64:128 = h1 c-dims, so a single K=128 matmul against
                # wo_sb sums both heads' contributions.
                tags = ["st0", "st1", "st2", "pv0", "pv1"]
                for lqc in range(8):
                    for s in range(4):
                        i = lqc * 4 + s
                        ops = psp.tile([P, D], F32, tag=tags[i % 5],
                                       name="ops")
                        nc.tensor.matmul(
                            ops, lhsT=mix_t[lqc][:, s * P:(s + 1) * P],
                            rhs=wo_sb, start=True, stop=True)
                        ob = obp.tile([P, D], BF16, tag=f"ob{i % 3}")
                        if i % 2 == 0:
                            nc.scalar.copy(ob, ops)
                        else:
                            nc.vector.tensor_copy(ob, ops)
                        nc.sync.dma_start(
                            out[lqc * 512 + s * P:lqc * 512 + (s + 1) * P, :],
                            ob)

    nc.compile()
    return nc


def get_nc():
    global _NC
    if _NC is None:
        _NC = build()
    return _NC


def make_in_maps(q, k, v, Wq, bq, Wk, bk, Wv, bv, Wo, bo):
    bf = ml_dtypes.bfloat16
    q = np.asarray(q, np.float32)
    k = np.asarray(k, np.float32)
    v = np.asarray(v, np.float32)
    Wq = np.asarray(Wq, np.float32)
    Wk = np.asarray(Wk, np.float32)
    Wv = np.asarray(Wv, np.float32)
    Wo = np.asarray(Wo, np.float32)
    bq = np.asarray(bq, np.float32)
    bk = np.asarray(bk, np.float32)
    bv = np.asarray(bv, np.float32)
    xts = {}
    for n in range(2):
        xts[n] = (np.ascontiguousarray(q[n].T).astype(bf),
                  np.ascontiguousarray(k[n].T).astype(bf),
                  np.ascontiguousarray(v[n].T).astype(bf))
    in_maps = []
    for c in range(8):
        n, hp = c // 4, c % 4
        sl = slice(P * hp, P * (hp + 1))
        xq, xk, xv = xts[n]
        in_maps.append({
            "xqt": xq, "xkt": xk, "xvt": xv,
            "wq": np.ascontiguousarray(Wq[:, sl]).astype(bf),
            "wk": np.ascontiguousarray(Wk[:, sl]).astype(bf),
            "wv": np.ascontiguousarray(Wv[:, sl]).astype(bf),
            "wo": np.ascontiguousarray(Wo[sl, :]),
            "bqs": (bq[sl] * 0.125).reshape(P, 1).astype(np.float32),
            "bkc": bk[sl].reshape(P, 1).astype(np.float32),
            "bvr": bv[sl].reshape(1, P).astype(bf),
        })
    return in_maps


def assemble(results, bo):
    bo = np.asarray(bo, np.float32)
    out = np.zeros((2, L, D), np.float32)
    for c in range(8):
        out[c // 4] += np.asarray(results[c]["out"], dtype=np.float32)
    out += bo[None, None, :]
    return out


def kernel(q, k, v, Wq, bq, Wk, bk, Wv, bv, Wo, bo):
    nc = get_nc()
    in_maps = make_in_maps(q, k, v, Wq, bq, Wk, bk, Wv, bv, Wo, bo)
    res = bass_utils.run_bass_kernel_spmd(nc, in_maps, core_ids=list(range(8)))
    return assemble(res.results, bo)


if __name__ == "__main__":
    build()
    print("build ok")


# revision 21
# speedup vs baseline: 1.4135x; 1.0084x over previous
"""Multi-head attention (q/k/v projections + softmax attention + out-projection)
on 8 Trainium2 NeuronCores.

Sharding: 16 (batch, head) units over 8 cores -> core c handles batch n = c//4
and head pair hp = c%4 (columns 128*hp : 128*hp+128 of the projections).
Per-core partial outputs (each pair's contribution to mix @ Wo) are summed on
host per batch, + bo.

Device kernel (per core), v6:
  - Host pre-transposes q[n],k[n],v[n] -> xT [512, 4096] and converts x and
    Wq/Wk/Wv to bf16 (halves input DMA; bf16 rhs streams 1 cy/row on the PE
    at any N, fixing the fp32r N=128 4x penalty on the V-projection).
  - Projections are interleaved into the first lq-window's attention blocks,
    so ScalarE exp starts a few us in instead of idling through a serial
    projection phase.
  - Attention runs on 512-wide lq windows (8 of them). Per window the two
    PV accumulators are [128,512] = 1 PSUM bank each, which frees budget
    for THREE [128, 2(kv),512] S^T tiles: the S->exp->free round-trip is
    no longer the pipeline pacer (with 2 buffers it was).
  - S^T in fp32r (K=64); exp on ScalarE over [128, 2x512] tiles (one per
    kv-chunk-pair and head); PV in bf16 accumulates mixT[c,lq] + sum(exp)
    via a ones-column in VP (transpose-free). PV runs one pair behind exp
    (software pipelining) so it never blocks the S/exp chain.
  - ~19% of est tiles (h1, odd pair, later windows) are computed on the DVE
    instead via a Schraudolph fast-exp (bitcast_bf16(int16(x*c1+c2)), one
    tensor_scalar op) - whole-tile offload spreads the approximation across
    kv so softmax dilutes it; this keeps ScalarE below the PE's pace.
  - softmax normalization: 1/sum(exp) from the PSUM row on DVE, GPSIMD
    partition-broadcast, one DVE multiply folds normalization into the
    PSUM->SBUF move of mix.
  - Out-projection is deferred to a tail phase: mix rows are h0|h1 c-dims,
    so one K=128 matmul against Wo sums both heads; output DMA'd as bf16
    (host sums partials in f32).
"""

import numpy as np
import ml_dtypes

import concourse.bacc as bacc
import concourse.mybir as mybir
import concourse.tile as tile
from concourse import bass_utils

P = 128
L = 4096
D = 512
F32 = mybir.dt.float32
F32R = mybir.dt.float32r
BF16 = mybir.dt.bfloat16
I16 = mybir.dt.int16
AF = mybir.ActivationFunctionType
ALU = mybir.AluOpType

_NC = None
V_OFFL_LQC = 2   # DVE fast-exp for h1/odd pairs in lq windows >= this


def build():
    nc = bacc.Bacc("TRN2", target_bir_lowering=False, debug=False)

    xqt = nc.dram_tensor("xqt", (D, L), BF16, kind="ExternalInput").ap()
    xkt = nc.dram_tensor("xkt", (D, L), BF16, kind="ExternalInput").ap()
    xvt = nc.dram_tensor("xvt", (D, L), BF16, kind="ExternalInput").ap()
    wq = nc.dram_tensor("wq", (D, P), BF16, kind="ExternalInput").ap()
    wk = nc.dram_tensor("wk", (D, P), BF16, kind="ExternalInput").ap()
    wv = nc.dram_tensor("wv", (D, P), BF16, kind="ExternalInput").ap()
    wo = nc.dram_tensor("wo", (P, D), F32R, kind="ExternalInput").ap()
    bqs = nc.dram_tensor("bqs", (P, 1), F32, kind="ExternalInput").ap()
    bkc = nc.dram_tensor("bkc", (P, 1), F32, kind="ExternalInput").ap()
    bvr = nc.dram_tensor("bvr", (1, P), BF16, kind="ExternalInput").ap()
    out = nc.dram_tensor("out", (L, D), BF16, kind="ExternalOutput").ap()

    with tile.TileContext(nc) as tc:
        with tc.tile_pool(name="const", bufs=1) as const, \
             tc.tile_pool(name="persist", bufs=1) as persist:
            xqv = xqt.rearrange("(o p) l -> p o l", p=P)
            xkv = xkt.rearrange("(o p) l -> p o l", p=P)
            xvv = xvt.rearrange("(o p) l -> p o l", p=P)

            wk_sb = const.tile([P, 4, P], BF16, tag="wk")
            nc.sync.dma_start(wk_sb, wk.rearrange("(o p) m -> p o m", p=P))
            wq_sb = const.tile([P, 4, P], BF16, tag="wq")
            nc.sync.dma_start(wq_sb, wq.rearrange("(o p) m -> p o m", p=P))
            bq_sb = const.tile([P, 1], F32, tag="bq")
            nc.sync.dma_start(bq_sb, bqs)
            bk_sb = const.tile([P, 1], F32, tag="bk")
            nc.sync.dma_start(bk_sb, bkc)
            wv_sb = const.tile([P, 4, P], BF16, tag="wv")
            nc.sync.dma_start(wv_sb, wv.rearrange("(o p) m -> p o m", p=P))
            bvr_sb = const.tile([1, P], BF16, tag="bvr")
            nc.sync.dma_start(bvr_sb, bvr)
            onesr = const.tile([1, P], BF16, tag="onesr")
            nc.scalar.activation(onesr, bvr_sb, AF.Identity,
                                 bias=1.0, scale=0.0)

            qpt_t = [persist.tile([P, 512], F32R, tag=f"qpt{c}",
                                  name=f"qpt{c}") for c in range(8)]
            kpt_t = [persist.tile([P, 512], F32R, tag=f"kpt{c}",
                                  name=f"kpt{c}") for c in range(8)]
            vp_t = [persist.tile([P, 4, 130], BF16, tag=f"vp{c}",
                                 name=f"vp{c}") for c in range(8)]
            mix_t = [persist.tile([P, 512], F32R, tag=f"mix{l}",
                                  name=f"mix{l}") for l in range(8)]
            ones_in = bq_sb[:, :, None].to_broadcast((P, 4, 1))
            for c in range(8):
                nc.vector.tensor_scalar(vp_t[c][:, :, 64:65], ones_in,
                                        0.0, 1.0, ALU.mult, ALU.add)
                nc.vector.tensor_scalar(vp_t[c][:, :, 129:130], ones_in,
                                        0.0, 1.0, ALU.mult, ALU.add)

            with tc.tile_pool(name="xs", bufs=2) as xs, \
                 tc.tile_pool(name="psp", bufs=1, space="PSUM") as psp, \
                 tc.tile_pool(name="esp", bufs=4) as esp, \
                 tc.tile_pool(name="bcp", bufs=1) as bcp, \
                 tc.tile_pool(name="obp", bufs=2) as obp:
                stc = [0]  # st tag rotation counter (3 tags x 2 banks)

                def st_tile(shape, name):
                    t = psp.tile(shape, F32, tag=f"st{stc[0] % 3}", name=name)
                    stc[0] += 1
                    return t

                def proj_k(ch):
                    sl = slice(ch * 512, (ch + 1) * 512)
                    xtk = xs.tile([P, 4, 512], BF16, tag="xtk")
                    nc.sync.dma_start(xtk, xkv[:, :, sl])
                    kps = st_tile([P, 512], "kps")
                    for dk in range(4):
                        nc.tensor.matmul(kps, lhsT=wk_sb[:, dk, :],
                                         rhs=xtk[:, dk, :],
                                         start=(dk == 0), stop=(dk == 3))
                    nc.vector.tensor_scalar(kpt_t[ch][:], kps, 1.0, bk_sb,
                                            ALU.mult, ALU.add)

                def proj_v(ch):
                    sl = slice(ch * 512, (ch + 1) * 512)
                    xtv = xs.tile([P, 4, 512], BF16, tag="xtv")
                    nc.gpsimd.dma_start(xtv, xvv[:, :, sl])
                    for js in range(4):
                        psv = st_tile([P, P], "psv")
                        for dk in range(4):
                            nc.tensor.matmul(psv,
                                             lhsT=xtv[:, dk, js * P:(js + 1) * P],
                                             rhs=wv_sb[:, dk, :],
                                             start=(dk == 0), stop=False)
                        nc.tensor.matmul(psv, lhsT=onesr, rhs=bvr_sb,
                                         start=False, stop=True)
                        nc.vector.tensor_copy(vp_t[ch][:, js, 0:64],
                                              psv[:, 0:64])
                        nc.vector.tensor_copy(vp_t[ch][:, js, 65:129],
                                              psv[:, 64:128])

                def proj_q(ch):
                    sl = slice(ch * 512, (ch + 1) * 512)
                    xtq = xs.tile([P, 4, 512], BF16, tag="xtq")
                    nc.gpsimd.dma_start(xtq, xqv[:, :, sl])
                    qps = st_tile([P, 512], "qps")
                    for dk in range(4):
                        nc.tensor.matmul(qps, lhsT=wq_sb[:, dk, :],
                                         rhs=xtq[:, dk, :],
                                         start=(dk == 0), stop=(dk == 3))
                    nc.vector.tensor_scalar(qpt_t[ch][:], qps, 0.125, bq_sb,
                                            ALU.mult, ALU.add)

                # prologue: just chunk 0; the rest interleaves with
                # attention (k/v chunk c lands 2 pairs before first use)
                proj_k(0)
                proj_q(0)
                proj_v(0)
                wo_sb = const.tile([P, D], F32R, tag="wo")
                nc.sync.dma_start(wo_sb, wo)

                # Schraudolph fast-exp in bf16: exp(x) ~=
                # bitcast_bf16(int16(x * 2^7/ln2 + (127*2^7 - 7.42)))
                SC1, SC2 = 184.6650292, 16249.0

                def emit_pv(pair, ests, pv_ps):
                    for h in range(2):
                        for r in range(2):
                            j = 2 * pair + r
                            nc.tensor.matmul(
                                pv_ps[h][0:65, :],
                                lhsT=vp_t[j // 4][:, j % 4,
                                                  h * 65:(h + 1) * 65],
                                rhs=ests[h][:, r, :],
                                start=(j == 0), stop=(j == 31))

                for lqc in range(8):
                    pv_ps = [psp.tile([P, 512], F32, tag=f"pv{h}",
                                      name=f"pv{h}") for h in range(2)]
                    prev = None
                    for b in range(8):
                        for pair in range(2 * b, 2 * b + 2):
                            ests = []
                            for h in range(2):
                                hb = h * 64
                                st = st_tile([P, 2, 512], "st")
                                for r in range(2):
                                    j = 2 * pair + r
                                    nc.tensor.matmul(
                                        st[:, r, :],
                                        lhsT=kpt_t[j // 4][hb:hb + 64,
                                                           (j % 4) * P:(j % 4 + 1) * P],
                                        rhs=qpt_t[lqc][hb:hb + 64, :],
                                        start=True, stop=True)
                                est = esp.tile([P, 2, 512], BF16,
                                               tag=f"est{h}")
                                if (lqc >= V_OFFL_LQC and h == 1
                                        and pair % 8 in (1, 3, 5)):
                                    nc.vector.tensor_scalar(
                                        est[:, :, :].bitcast(I16),
                                        st, SC1, SC2, ALU.mult, ALU.add)
                                else:
                                    nc.scalar.activation(est, st, AF.Exp)
                                ests.append(est)
                            if prev is not None:
                                emit_pv(prev[0], prev[1], pv_ps)
                            prev = (pair, ests)
                            # interleaved projections, half-chunk granular:
                            # k(c)/v(c) emitted at pairs 2(c-1), 2(c-1)+1
                            if lqc == 0 and pair < 14:
                                if pair % 2 == 0:
                                    proj_k(pair // 2 + 1)
                                else:
                                    proj_v(pair // 2 + 1)
                            elif lqc == 0 and pair == 14:
                                proj_q(1)
                            elif lqc == 0 and pair == 15:
                                proj_q(2)
                        if lqc in (1, 2) and b in (0, 4):
                            qch = 2 * lqc + 1 + (b // 4)
                            if 2 < qch < 8:
                                proj_q(qch)
                        if lqc == 3 and b == 0:
                            proj_q(7)
                    emit_pv(prev[0], prev[1], pv_ps)
                    # free pv banks fast: raw copy to SBUF; normalization
                    # (recip/broadcast/mult into mix) happens lazily and
                    # overlaps the next window's attention.
                    for h in range(2):
                        raw = bcp.tile([P, 512], F32, tag=f"raw{h}", bufs=2)
                        nc.vector.tensor_copy(raw[0:65, :], pv_ps[h][0:65, :])
                        rr = bcp.tile([1, 512], F32, tag=f"rr{h}", bufs=2)
                        nc.vector.reciprocal(rr, raw[64:65, :])
                        bc = bcp.tile([P, 512], F32, tag=f"bc{h}", bufs=2)
                        nc.gpsimd.partition_broadcast(bc, rr)
                        nc.vector.tensor_tensor(
                            mix_t[lqc][h * 64:(h + 1) * 64, :],
                            raw[0:64, :], bc[0:64, :], ALU.mult)

                # tail: out-projection. mix rows 0:64 = h0 c-dims,
                # 64:128 = h1 c-dims, so a single K=128 matmul against
                # wo_sb sums both heads' contributions.
                tags = ["st0", "st1", "st2", "pv0", "pv1"]
                for lqc in range(8):
                    for s in range(4):
                        i = lqc * 4 + s
                        ops = psp.tile([P, D], F32, tag=tags[i % 5],
                                       name="ops")
                        nc.tensor.matmul(
                            ops, lhsT=mix_t[lqc][:, s * P:(s + 1) * P],
                            rhs=wo_sb, start=True, stop=True)
                        ob = obp.tile([P, D], BF16, tag=f"ob{i % 3}")
                        if i % 2 == 0:
                            nc.scalar.copy(ob, ops)
                        else:
                            nc.vector.tensor_copy(ob, ops)
                        nc.sync.dma_start(
                            out[lqc * 512 + s * P:lqc * 512 + (s + 1) * P, :],
                            ob)

    nc.compile()
    return nc


def get_nc():
    global _NC
    if _NC is None:
        _NC = build()
    return _NC


def make_in_maps(q, k, v, Wq, bq, Wk, bk, Wv, bv, Wo, bo):
    bf = ml_dtypes.bfloat16
    q = np.asarray(q, np.float32)
    k = np.asarray(k, np.float32)
    v = np.asarray(v, np.float32)
    Wq = np.asarray(Wq, np.float32)
    Wk = np.asarray(Wk, np.float32)
    Wv = np.asarray(Wv, np.float32)
    Wo = np.asarray(Wo, np.float32)
    bq = np.asarray(bq, np.float32)
    bk = np.asarray(bk, np.float32)
    bv = np.asarray(bv, np.float32)
    xts = {}
    for n in range(2):
        xts[n] = (np.ascontiguousarray(q[n].T).astype(bf),
                  np.ascontiguousarray(k[n].T).astype(bf),
                  np.ascontiguousarray(v[n].T).astype(bf))
    in_maps = []
    for c in range(8):
        n, hp = c // 4, c % 4
        sl = slice(P * hp, P * (hp + 1))
        xq, xk, xv = xts[n]
        in_maps.append({
            "xqt": xq, "xkt": xk, "xvt": xv,
            "wq": np.ascontiguousarray(Wq[:, sl]).astype(bf),
            "wk": np.ascontiguousarray(Wk[:, sl]).astype(bf),
            "wv": np.ascontiguousarray(Wv[:, sl]).astype(bf),
            "wo": np.ascontiguousarray(Wo[sl, :]),
            "bqs": (bq[sl] * 0.125).reshape(P, 1).astype(np.float32),
            "bkc": bk[sl].reshape(P, 1).astype(np.float32),
            "bvr": bv[sl].reshape(1, P).astype(bf),
        })
    return in_maps


def assemble(results, bo):
    bo = np.asarray(bo, np.float32)
    out = np.zeros((2, L, D), np.float32)
    for c in range(8):
        out[c // 4] += np.asarray(results[c]["out"], dtype=np.float32)
    out += bo[None, None, :]
    return out


def kernel(q, k, v, Wq, bq, Wk, bk, Wv, bv, Wo, bo):
    nc = get_nc()
    in_maps = make_in_maps(q, k, v, Wq, bq, Wk, bk, Wv, bv, Wo, bo)
    res = bass_utils.run_bass_kernel_spmd(nc, in_maps, core_ids=list(range(8)))
    return assemble(res.results, bo)


if __name__ == "__main__":
    build()
    print("build ok")


# revision 22
# speedup vs baseline: 1.4533x; 1.0281x over previous
"""Multi-head attention (q/k/v projections + softmax attention + out-projection)
on 8 Trainium2 NeuronCores.

Sharding: 16 (batch, head) units over 8 cores -> core c handles batch n = c//4
and head pair hp = c%4 (columns 128*hp : 128*hp+128 of the projections).
Per-core partial outputs (each pair's contribution to mix @ Wo) are summed on
host per batch, + bo.

Device kernel (per core), v6:
  - Host pre-transposes q[n],k[n],v[n] -> xT [512, 4096] and converts x and
    Wq/Wk/Wv to bf16 (halves input DMA; bf16 rhs streams 1 cy/row on the PE
    at any N, fixing the fp32r N=128 4x penalty on the V-projection).
  - Projections are interleaved into the first lq-window's attention blocks,
    so ScalarE exp starts a few us in instead of idling through a serial
    projection phase.
  - Attention runs on 512-wide lq windows (8 of them). Per window the two
    PV accumulators are [128,512] = 1 PSUM bank each, which frees budget
    for THREE [128, 2(kv),512] S^T tiles: the S->exp->free round-trip is
    no longer the pipeline pacer (with 2 buffers it was).
  - S^T in fp32r (K=64); exp on ScalarE over [128, 2x512] tiles (one per
    kv-chunk-pair and head); PV in bf16 accumulates mixT[c,lq] + sum(exp)
    via a ones-column in VP (transpose-free). PV runs one pair behind exp
    (software pipelining) so it never blocks the S/exp chain.
  - ~19% of est tiles (h1, odd pair, later windows) are computed on the DVE
    instead via a Schraudolph fast-exp (bitcast_bf16(int16(x*c1+c2)), one
    tensor_scalar op) - whole-tile offload spreads the approximation across
    kv so softmax dilutes it; this keeps ScalarE below the PE's pace.
  - softmax normalization: 1/sum(exp) from the PSUM row on DVE, GPSIMD
    partition-broadcast, one DVE multiply folds normalization into the
    PSUM->SBUF move of mix.
  - Out-projection is deferred to a tail phase: mix rows are h0|h1 c-dims,
    so one K=128 matmul against Wo sums both heads; output DMA'd as bf16
    (host sums partials in f32).
"""

import numpy as np
import ml_dtypes

import concourse.bacc as bacc
import concourse.mybir as mybir
import concourse.tile as tile
from concourse import bass_utils

P = 128
L = 4096
D = 512
F32 = mybir.dt.float32
F32R = mybir.dt.float32r
BF16 = mybir.dt.bfloat16
I16 = mybir.dt.int16
AF = mybir.ActivationFunctionType
ALU = mybir.AluOpType

_NC = None
V_OFFL_LQC = 2   # DVE fast-exp for h1/odd pairs in lq windows >= this


def build():
    nc = bacc.Bacc("TRN2", target_bir_lowering=False, debug=False)

    xqt = nc.dram_tensor("xqt", (D, L), BF16, kind="ExternalInput").ap()
    xkt = nc.dram_tensor("xkt", (D, L), BF16, kind="ExternalInput").ap()
    xvt = nc.dram_tensor("xvt", (D, L), BF16, kind="ExternalInput").ap()
    wq = nc.dram_tensor("wq", (D, P), BF16, kind="ExternalInput").ap()
    wk = nc.dram_tensor("wk", (D, P), BF16, kind="ExternalInput").ap()
    wv = nc.dram_tensor("wv", (D, P), BF16, kind="ExternalInput").ap()
    wo = nc.dram_tensor("wo", (P, D), F32R, kind="ExternalInput").ap()
    bqs = nc.dram_tensor("bqs", (P, 1), F32, kind="ExternalInput").ap()
    bkc = nc.dram_tensor("bkc", (P, 1), F32, kind="ExternalInput").ap()
    bvr = nc.dram_tensor("bvr", (1, P), BF16, kind="ExternalInput").ap()
    out = nc.dram_tensor("out", (L, D), BF16, kind="ExternalOutput").ap()

    with tile.TileContext(nc) as tc:
        with tc.tile_pool(name="const", bufs=1) as const, \
             tc.tile_pool(name="persist", bufs=1) as persist:
            xqv = xqt.rearrange("(o p) l -> p o l", p=P)
            xkv = xkt.rearrange("(o p) l -> p o l", p=P)
            xvv = xvt.rearrange("(o p) l -> p o l", p=P)

            wk_sb = const.tile([P, 4, P], BF16, tag="wk")
            nc.sync.dma_start(wk_sb, wk.rearrange("(o p) m -> p o m", p=P))
            wq_sb = const.tile([P, 4, P], BF16, tag="wq")
            nc.sync.dma_start(wq_sb, wq.rearrange("(o p) m -> p o m", p=P))
            bq_sb = const.tile([P, 1], F32, tag="bq")
            nc.sync.dma_start(bq_sb, bqs)
            bk_sb = const.tile([P, 1], F32, tag="bk")
            nc.sync.dma_start(bk_sb, bkc)
            wv_sb = const.tile([P, 4, P], BF16, tag="wv")
            nc.sync.dma_start(wv_sb, wv.rearrange("(o p) m -> p o m", p=P))
            bvr_sb = const.tile([1, P], BF16, tag="bvr")
            nc.sync.dma_start(bvr_sb, bvr)
            onesr = const.tile([1, P], BF16, tag="onesr")
            nc.scalar.activation(onesr, bvr_sb, AF.Identity,
                                 bias=1.0, scale=0.0)

            qpt_t = [persist.tile([P, 512], F32R, tag=f"qpt{c}",
                                  name=f"qpt{c}") for c in range(8)]
            kpt_t = [persist.tile([P, 512], F32R, tag=f"kpt{c}",
                                  name=f"kpt{c}") for c in range(8)]
            vp_t = [persist.tile([P, 4, 130], BF16, tag=f"vp{c}",
                                 name=f"vp{c}") for c in range(8)]
            mix_t = [persist.tile([P, 512], F32R, tag=f"mix{l}",
                                  name=f"mix{l}") for l in range(8)]
            ones_in = bq_sb[:, :, None].to_broadcast((P, 4, 1))
            for c in range(8):
                nc.vector.tensor_scalar(vp_t[c][:, :, 64:65], ones_in,
                                        0.0, 1.0, ALU.mult, ALU.add)
                nc.vector.tensor_scalar(vp_t[c][:, :, 129:130], ones_in,
                                        0.0, 1.0, ALU.mult, ALU.add)

            with tc.tile_pool(name="xs", bufs=2) as xs, \
                 tc.tile_pool(name="psp", bufs=1, space="PSUM") as psp, \
                 tc.tile_pool(name="esp", bufs=4) as esp, \
                 tc.tile_pool(name="bcp", bufs=1) as bcp, \
                 tc.tile_pool(name="obp", bufs=2) as obp:
                stc = [0]  # st tag rotation counter (3 tags x 2 banks)

                def st_tile(shape, name):
                    t = psp.tile(shape, F32, tag=f"st{stc[0] % 3}", name=name)
                    stc[0] += 1
                    return t

                def proj_k(ch):
                    sl = slice(ch * 512, (ch + 1) * 512)
                    xtk = xs.tile([P, 4, 512], BF16, tag="xtk")
                    nc.sync.dma_start(xtk, xkv[:, :, sl])
                    kps = st_tile([P, 512], "kps")
                    for dk in range(4):
                        nc.tensor.matmul(kps, lhsT=wk_sb[:, dk, :],
                                         rhs=xtk[:, dk, :],
                                         start=(dk == 0), stop=(dk == 3))
                    nc.vector.tensor_scalar(kpt_t[ch][:], kps, 1.0, bk_sb,
                                            ALU.mult, ALU.add)

                def proj_v(ch):
                    sl = slice(ch * 512, (ch + 1) * 512)
                    xtv = xs.tile([P, 4, 512], BF16, tag="xtv")
                    nc.gpsimd.dma_start(xtv, xvv[:, :, sl])
                    for js in range(4):
                        psv = st_tile([P, P], "psv")
                        for dk in range(4):
                            nc.tensor.matmul(psv,
                                             lhsT=xtv[:, dk, js * P:(js + 1) * P],
                                             rhs=wv_sb[:, dk, :],
                                             start=(dk == 0), stop=False)
                        nc.tensor.matmul(psv, lhsT=onesr, rhs=bvr_sb,
                                         start=False, stop=True)
                        nc.vector.tensor_copy(vp_t[ch][:, js, 0:64],
                                              psv[:, 0:64])
                        nc.vector.tensor_copy(vp_t[ch][:, js, 65:129],
                                              psv[:, 64:128])

                def proj_q(ch):
                    sl = slice(ch * 512, (ch + 1) * 512)
                    xtq = xs.tile([P, 4, 512], BF16, tag="xtq")
                    nc.gpsimd.dma_start(xtq, xqv[:, :, sl])
                    qps = st_tile([P, 512], "qps")
                    for dk in range(4):
                        nc.tensor.matmul(qps, lhsT=wq_sb[:, dk, :],
                                         rhs=xtq[:, dk, :],
                                         start=(dk == 0), stop=(dk == 3))
                    nc.vector.tensor_scalar(qpt_t[ch][:], qps, 0.125, bq_sb,
                                            ALU.mult, ALU.add)

                # prologue: just chunk 0; the rest interleaves with
                # attention (k/v chunk c lands 2 pairs before first use)
                proj_k(0)
                proj_q(0)
                proj_v(0)
                wo_sb = const.tile([P, D], F32R, tag="wo")
                nc.sync.dma_start(wo_sb, wo)

                # Schraudolph fast-exp in bf16: exp(x) ~=
                # bitcast_bf16(int16(x * 2^7/ln2 + (127*2^7 - 7.42)))
                SC1, SC2 = 184.6650292, 16249.0

                def emit_pv(pair, ests, pv_ps):
                    for h in range(2):
                        for r in range(2):
                            j = 2 * pair + r
                            nc.tensor.matmul(
                                pv_ps[h][0:65, :],
                                lhsT=vp_t[j // 4][:, j % 4,
                                                  h * 65:(h + 1) * 65],
                                rhs=ests[h][:, r, :],
                                start=(j == 0), stop=(j == 31))

                for lqc in range(8):
                    pv_ps = [psp.tile([P, 512], F32, tag=f"pv{h}",
                                      name=f"pv{h}") for h in range(2)]
                    prev = None
                    for b in range(8):
                        for pair in range(2 * b, 2 * b + 2):
                            ests = []
                            for h in range(2):
                                hb = h * 64
                                st = st_tile([P, 2, 512], "st")
                                for r in range(2):
                                    j = 2 * pair + r
                                    nc.tensor.matmul(
                                        st[:, r, :],
                                        lhsT=kpt_t[j // 4][hb:hb + 64,
                                                           (j % 4) * P:(j % 4 + 1) * P],
                                        rhs=qpt_t[lqc][hb:hb + 64, :],
                                        start=True, stop=True)
                                est = esp.tile([P, 2, 512], BF16,
                                               tag=f"est{h}")
                                if (lqc >= V_OFFL_LQC and h == 1
                                        and pair % 2 == 1):
                                    nc.vector.tensor_scalar(
                                        est[:, :, :].bitcast(I16),
                                        st, SC1, SC2, ALU.mult, ALU.add)
                                else:
                                    nc.scalar.activation(est, st, AF.Exp)
                                ests.append(est)
                            if prev is not None:
                                emit_pv(prev[0], prev[1], pv_ps)
                            prev = (pair, ests)
                            # interleaved projections, half-chunk granular:
                            # k(c)/v(c) emitted at pairs 2(c-1), 2(c-1)+1
                            if lqc == 0 and pair < 14:
                                if pair % 2 == 0:
                                    proj_k(pair // 2 + 1)
                                else:
                                    proj_v(pair // 2 + 1)
                            elif lqc == 0 and pair == 14:
                                proj_q(1)
                            elif lqc == 0 and pair == 15:
                                proj_q(2)
                        if lqc in (1, 2) and b in (0, 4):
                            qch = 2 * lqc + 1 + (b // 4)
                            if 2 < qch < 8:
                                proj_q(qch)
                        if lqc == 3 and b == 0:
                            proj_q(7)
                    emit_pv(prev[0], prev[1], pv_ps)
                    # free pv banks fast: raw copy to SBUF; normalization
                    # (recip/broadcast/mult into mix) happens lazily and
                    # overlaps the next window's attention.
                    for h in range(2):
                        raw = bcp.tile([P, 512], F32, tag=f"raw{h}", bufs=2)
                        nc.vector.tensor_copy(raw[0:65, :], pv_ps[h][0:65, :])
                        rr = bcp.tile([1, 512], F32, tag=f"rr{h}", bufs=2)
                        nc.vector.reciprocal(rr, raw[64:65, :])
                        bc = bcp.tile([P, 512], F32, tag=f"bc{h}", bufs=2)
                        nc.gpsimd.partition_broadcast(bc, rr)
                        nc.vector.tensor_tensor(
                            mix_t[lqc][h * 64:(h + 1) * 64, :],
                            raw[0:64, :], bc[0:64, :], ALU.mult)

                # tail: out-projection. mix rows 0:64 = h0 c-dims,
                # 64:128 = h1 c-dims, so a single K=128 matmul against
                # wo_sb sums both heads' contributions.
                tags = ["st0", "st1", "st2", "pv0", "pv1"]
                for lqc in range(8):
                    for s in range(4):
                        i = lqc * 4 + s
                        ops = psp.tile([P, D], F32, tag=tags[i % 5],
                                       name="ops")
                        nc.tensor.matmul(
                            ops, lhsT=mix_t[lqc][:, s * P:(s + 1) * P],
                            rhs=wo_sb, start=True, stop=True)
                        ob = obp.tile([P, D], BF16, tag=f"ob{i % 3}")
                        if i % 2 == 0:
                            nc.scalar.copy(ob, ops)
                        else:
                            nc.vector.tensor_copy(ob, ops)
                        nc.sync.dma_start(
                            out[lqc * 512 + s * P:lqc * 512 + (s + 1) * P, :],
                            ob)

    nc.compile()
    return nc


def get_nc():
    global _NC
    if _NC is None:
        _NC = build()
    return _NC


def make_in_maps(q, k, v, Wq, bq, Wk, bk, Wv, bv, Wo, bo):
    bf = ml_dtypes.bfloat16
    q = np.asarray(q, np.float32)
    k = np.asarray(k, np.float32)
    v = np.asarray(v, np.float32)
    Wq = np.asarray(Wq, np.float32)
    Wk = np.asarray(Wk, np.float32)
    Wv = np.asarray(Wv, np.float32)
    Wo = np.asarray(Wo, np.float32)
    bq = np.asarray(bq, np.float32)
    bk = np.asarray(bk, np.float32)
    bv = np.asarray(bv, np.float32)
    xts = {}
    for n in range(2):
        xts[n] = (np.ascontiguousarray(q[n].T).astype(bf),
                  np.ascontiguousarray(k[n].T).astype(bf),
                  np.ascontiguousarray(v[n].T).astype(bf))
    in_maps = []
    for c in range(8):
        n, hp = c // 4, c % 4
        sl = slice(P * hp, P * (hp + 1))
        xq, xk, xv = xts[n]
        in_maps.append({
            "xqt": xq, "xkt": xk, "xvt": xv,
            "wq": np.ascontiguousarray(Wq[:, sl]).astype(bf),
            "wk": np.ascontiguousarray(Wk[:, sl]).astype(bf),
            "wv": np.ascontiguousarray(Wv[:, sl]).astype(bf),
            "wo": np.ascontiguousarray(Wo[sl, :]),
            "bqs": (bq[sl] * 0.125).reshape(P, 1).astype(np.float32),
            "bkc": bk[sl].reshape(P, 1).astype(np.float32),
            "bvr": bv[sl].reshape(1, P).astype(bf),
        })
    return in_maps


def assemble(results, bo):
    bo = np.asarray(bo, np.float32)
    out = np.zeros((2, L, D), np.float32)
    for c in range(8):
        out[c // 4] += np.asarray(results[c]["out"], dtype=np.float32)
    out += bo[None, None, :]
    return out


def kernel(q, k, v, Wq, bq, Wk, bk, Wv, bv, Wo, bo):
    nc = get_nc()
    in_maps = make_in_maps(q, k, v, Wq, bq, Wk, bk, Wv, bv, Wo, bo)
    res = bass_utils.run_bass_kernel_spmd(nc, in_maps, core_ids=list(range(8)))
    return assemble(res.results, bo)


if __name__ == "__main__":
    build()
    print("build ok")


# revision 23
# speedup vs baseline: 1.4739x; 1.0142x over previous
"""Multi-head attention (q/k/v projections + softmax attention + out-projection)
on 8 Trainium2 NeuronCores.

Sharding: 16 (batch, head) units over 8 cores -> core c handles batch n = c//4
and head pair hp = c%4 (columns 128*hp : 128*hp+128 of the projections).
Per-core partial outputs (each pair's contribution to mix @ Wo) are summed on
host per batch, + bo.

Device kernel (per core), v6:
  - Host pre-transposes q[n],k[n],v[n] -> xT [512, 4096] and converts x and
    Wq/Wk/Wv to bf16 (halves input DMA; bf16 rhs streams 1 cy/row on the PE
    at any N, fixing the fp32r N=128 4x penalty on the V-projection).
  - Projections are interleaved into the first lq-window's attention blocks,
    so ScalarE exp starts a few us in instead of idling through a serial
    projection phase.
  - Attention runs on 512-wide lq windows (8 of them). Per window the two
    PV accumulators are [128,512] = 1 PSUM bank each, which frees budget
    for THREE [128, 2(kv),512] S^T tiles: the S->exp->free round-trip is
    no longer the pipeline pacer (with 2 buffers it was).
  - S^T in fp32r (K=64); exp on ScalarE over [128, 2x512] tiles (one per
    kv-chunk-pair and head); PV in bf16 accumulates mixT[c,lq] + sum(exp)
    via a ones-column in VP (transpose-free). PV runs one pair behind exp
    (software pipelining) so it never blocks the S/exp chain.
  - ~19% of est tiles (h1, odd pair, later windows) are computed on the DVE
    instead via a Schraudolph fast-exp (bitcast_bf16(int16(x*c1+c2)), one
    tensor_scalar op) - whole-tile offload spreads the approximation across
    kv so softmax dilutes it; this keeps ScalarE below the PE's pace.
  - softmax normalization: 1/sum(exp) from the PSUM row on DVE, GPSIMD
    partition-broadcast, one DVE multiply folds normalization into the
    PSUM->SBUF move of mix.
  - Out-projection is deferred to a tail phase: mix rows are h0|h1 c-dims,
    so one K=128 matmul against Wo sums both heads; output DMA'd as bf16
    (host sums partials in f32).
"""

import numpy as np
import ml_dtypes

import concourse.bacc as bacc
import concourse.mybir as mybir
import concourse.tile as tile
from concourse import bass_utils

P = 128
L = 4096
D = 512
F32 = mybir.dt.float32
F32R = mybir.dt.float32r
BF16 = mybir.dt.bfloat16
I16 = mybir.dt.int16
AF = mybir.ActivationFunctionType
ALU = mybir.AluOpType

_NC = None
V_OFFL_LQC = 1   # DVE fast-exp for h1/odd pairs in lq windows >= this


def build():
    nc = bacc.Bacc("TRN2", target_bir_lowering=False, debug=False)

    xqt = nc.dram_tensor("xqt", (D, L), BF16, kind="ExternalInput").ap()
    xkt = nc.dram_tensor("xkt", (D, L), BF16, kind="ExternalInput").ap()
    xvt = nc.dram_tensor("xvt", (D, L), BF16, kind="ExternalInput").ap()
    wq = nc.dram_tensor("wq", (D, P), BF16, kind="ExternalInput").ap()
    wk = nc.dram_tensor("wk", (D, P), BF16, kind="ExternalInput").ap()
    wv = nc.dram_tensor("wv", (D, P), BF16, kind="ExternalInput").ap()
    wo = nc.dram_tensor("wo", (P, D), F32R, kind="ExternalInput").ap()
    bqs = nc.dram_tensor("bqs", (P, 1), F32, kind="ExternalInput").ap()
    bkc = nc.dram_tensor("bkc", (P, 1), F32, kind="ExternalInput").ap()
    bvr = nc.dram_tensor("bvr", (1, P), BF16, kind="ExternalInput").ap()
    out = nc.dram_tensor("out", (L, D), BF16, kind="ExternalOutput").ap()

    with tile.TileContext(nc) as tc:
        with tc.tile_pool(name="const", bufs=1) as const, \
             tc.tile_pool(name="persist", bufs=1) as persist:
            xqv = xqt.rearrange("(o p) l -> p o l", p=P)
            xkv = xkt.rearrange("(o p) l -> p o l", p=P)
            xvv = xvt.rearrange("(o p) l -> p o l", p=P)

            wk_sb = const.tile([P, 4, P], BF16, tag="wk")
            nc.sync.dma_start(wk_sb, wk.rearrange("(o p) m -> p o m", p=P))
            wq_sb = const.tile([P, 4, P], BF16, tag="wq")
            nc.sync.dma_start(wq_sb, wq.rearrange("(o p) m -> p o m", p=P))
            bq_sb = const.tile([P, 1], F32, tag="bq")
            nc.sync.dma_start(bq_sb, bqs)
            bk_sb = const.tile([P, 1], F32, tag="bk")
            nc.sync.dma_start(bk_sb, bkc)
            wv_sb = const.tile([P, 4, P], BF16, tag="wv")
            nc.sync.dma_start(wv_sb, wv.rearrange("(o p) m -> p o m", p=P))
            bvr_sb = const.tile([1, P], BF16, tag="bvr")
            nc.sync.dma_start(bvr_sb, bvr)
            onesr = const.tile([1, P], BF16, tag="onesr")
            nc.scalar.activation(onesr, bvr_sb, AF.Identity,
                                 bias=1.0, scale=0.0)

            qpt_t = [persist.tile([P, 512], F32R, tag=f"qpt{c}",
                                  name=f"qpt{c}") for c in range(8)]
            kpt_t = [persist.tile([P, 512], F32R, tag=f"kpt{c}",
                                  name=f"kpt{c}") for c in range(8)]
            vp_t = [persist.tile([P, 4, 130], BF16, tag=f"vp{c}",
                                 name=f"vp{c}") for c in range(8)]
            mix_t = [persist.tile([P, 512], F32R, tag=f"mix{l}",
                                  name=f"mix{l}") for l in range(8)]
            ones_in = bq_sb[:, :, None].to_broadcast((P, 4, 1))
            for c in range(8):
                nc.vector.tensor_scalar(vp_t[c][:, :, 64:65], ones_in,
                                        0.0, 1.0, ALU.mult, ALU.add)
                nc.vector.tensor_scalar(vp_t[c][:, :, 129:130], ones_in,
                                        0.0, 1.0, ALU.mult, ALU.add)

            with tc.tile_pool(name="xs", bufs=2) as xs, \
                 tc.tile_pool(name="psp", bufs=1, space="PSUM") as psp, \
                 tc.tile_pool(name="esp", bufs=4) as esp, \
                 tc.tile_pool(name="bcp", bufs=1) as bcp, \
                 tc.tile_pool(name="obp", bufs=2) as obp:
                stc = [0]  # st tag rotation counter (3 tags x 2 banks)

                def st_tile(shape, name):
                    t = psp.tile(shape, F32, tag=f"st{stc[0] % 3}", name=name)
                    stc[0] += 1
                    return t

                def proj_k(ch):
                    sl = slice(ch * 512, (ch + 1) * 512)
                    xtk = xs.tile([P, 4, 512], BF16, tag="xtk")
                    nc.sync.dma_start(xtk, xkv[:, :, sl])
                    kps = st_tile([P, 512], "kps")
                    for dk in range(4):
                        nc.tensor.matmul(kps, lhsT=wk_sb[:, dk, :],
                                         rhs=xtk[:, dk, :],
                                         start=(dk == 0), stop=(dk == 3))
                    nc.vector.tensor_scalar(kpt_t[ch][:], kps, 1.0, bk_sb,
                                            ALU.mult, ALU.add)

                def proj_v(ch):
                    sl = slice(ch * 512, (ch + 1) * 512)
                    xtv = xs.tile([P, 4, 512], BF16, tag="xtv")
                    nc.gpsimd.dma_start(xtv, xvv[:, :, sl])
                    for js in range(4):
                        psv = st_tile([P, P], "psv")
                        for dk in range(4):
                            nc.tensor.matmul(psv,
                                             lhsT=xtv[:, dk, js * P:(js + 1) * P],
                                             rhs=wv_sb[:, dk, :],
                                             start=(dk == 0), stop=False)
                        nc.tensor.matmul(psv, lhsT=onesr, rhs=bvr_sb,
                                         start=False, stop=True)
                        nc.vector.tensor_copy(vp_t[ch][:, js, 0:64],
                                              psv[:, 0:64])
                        nc.vector.tensor_copy(vp_t[ch][:, js, 65:129],
                                              psv[:, 64:128])

                def proj_q(ch):
                    sl = slice(ch * 512, (ch + 1) * 512)
                    xtq = xs.tile([P, 4, 512], BF16, tag="xtq")
                    nc.gpsimd.dma_start(xtq, xqv[:, :, sl])
                    qps = st_tile([P, 512], "qps")
                    for dk in range(4):
                        nc.tensor.matmul(qps, lhsT=wq_sb[:, dk, :],
                                         rhs=xtq[:, dk, :],
                                         start=(dk == 0), stop=(dk == 3))
                    nc.vector.tensor_scalar(qpt_t[ch][:], qps, 0.125, bq_sb,
                                            ALU.mult, ALU.add)

                # prologue: just chunk 0; the rest interleaves with
                # attention (k/v chunk c lands 2 pairs before first use)
                proj_k(0)
                proj_q(0)
                proj_v(0)
                wo_sb = const.tile([P, D], F32R, tag="wo")
                nc.sync.dma_start(wo_sb, wo)

                # Schraudolph fast-exp in bf16: exp(x) ~=
                # bitcast_bf16(int16(x * 2^7/ln2 + (127*2^7 - 7.42)))
                SC1, SC2 = 184.6650292, 16249.0

                def emit_pv(pair, ests, pv_ps):
                    for h in range(2):
                        for r in range(2):
                            j = 2 * pair + r
                            nc.tensor.matmul(
                                pv_ps[h][0:65, :],
                                lhsT=vp_t[j // 4][:, j % 4,
                                                  h * 65:(h + 1) * 65],
                                rhs=ests[h][:, r, :],
                                start=(j == 0), stop=(j == 31))

                for lqc in range(8):
                    pv_ps = [psp.tile([P, 512], F32, tag=f"pv{h}",
                                      name=f"pv{h}") for h in range(2)]
                    prev = None
                    for b in range(8):
                        for pair in range(2 * b, 2 * b + 2):
                            ests = []
                            for h in range(2):
                                hb = h * 64
                                st = st_tile([P, 2, 512], "st")
                                for r in range(2):
                                    j = 2 * pair + r
                                    nc.tensor.matmul(
                                        st[:, r, :],
                                        lhsT=kpt_t[j // 4][hb:hb + 64,
                                                           (j % 4) * P:(j % 4 + 1) * P],
                                        rhs=qpt_t[lqc][hb:hb + 64, :],
                                        start=True, stop=True)
                                est = esp.tile([P, 2, 512], BF16,
                                               tag=f"est{h}")
                                if (lqc >= V_OFFL_LQC and h == 1
                                        and pair % 2 == 1):
                                    nc.vector.tensor_scalar(
                                        est[:, :, :].bitcast(I16),
                                        st, SC1, SC2, ALU.mult, ALU.add)
                                else:
                                    nc.scalar.activation(est, st, AF.Exp)
                                ests.append(est)
                            if prev is not None:
                                emit_pv(prev[0], prev[1], pv_ps)
                            prev = (pair, ests)
                            # interleaved projections, half-chunk granular:
                            # k(c)/v(c) emitted at pairs 2(c-1), 2(c-1)+1
                            if lqc == 0 and pair < 14:
                                if pair % 2 == 0:
                                    proj_k(pair // 2 + 1)
                                else:
                                    proj_v(pair // 2 + 1)
                            elif lqc == 0 and pair == 14:
                                proj_q(1)
                            elif lqc == 0 and pair == 15:
                                proj_q(2)
                        if lqc in (1, 2) and b in (0, 4):
                            qch = 2 * lqc + 1 + (b // 4)
                            if 2 < qch < 8:
                                proj_q(qch)
                        if lqc == 3 and b == 0:
                            proj_q(7)
                    emit_pv(prev[0], prev[1], pv_ps)
                    # free pv banks fast: raw copy to SBUF; normalization
                    # (recip/broadcast/mult into mix) happens lazily and
                    # overlaps the next window's attention.
                    for h in range(2):
                        raw = bcp.tile([P, 512], F32, tag=f"raw{h}", bufs=2)
                        nc.vector.tensor_copy(raw[0:65, :], pv_ps[h][0:65, :])
                        rr = bcp.tile([1, 512], F32, tag=f"rr{h}", bufs=2)
                        nc.vector.reciprocal(rr, raw[64:65, :])
                        bc = bcp.tile([P, 512], F32, tag=f"bc{h}", bufs=2)
                        nc.gpsimd.partition_broadcast(bc, rr)
                        nc.vector.tensor_tensor(
                            mix_t[lqc][h * 64:(h + 1) * 64, :],
                            raw[0:64, :], bc[0:64, :], ALU.mult)

                # tail: out-projection. mix rows 0:64 = h0 c-dims,
                # 64:128 = h1 c-dims, so a single K=128 matmul against
                # wo_sb sums both heads' contributions.
                tags = ["st0", "st1", "st2", "pv0", "pv1"]
                for lqc in range(8):
                    for s in range(4):
                        i = lqc * 4 + s
                        ops = psp.tile([P, D], F32, tag=tags[i % 5],
                                       name="ops")
                        nc.tensor.matmul(
                            ops, lhsT=mix_t[lqc][:, s * P:(s + 1) * P],
                            rhs=wo_sb, start=True, stop=True)
                        ob = obp.tile([P, D], BF16, tag=f"ob{i % 3}")
                        if i % 2 == 0:
                            nc.scalar.copy(ob, ops)
                        else:
                            nc.vector.tensor_copy(ob, ops)
                        nc.sync.dma_start(
                            out[lqc * 512 + s * P:lqc * 512 + (s + 1) * P, :],
                            ob)

    nc.compile()
    return nc


def get_nc():
    global _NC
    if _NC is None:
        _NC = build()
    return _NC


def make_in_maps(q, k, v, Wq, bq, Wk, bk, Wv, bv, Wo, bo):
    bf = ml_dtypes.bfloat16
    q = np.asarray(q, np.float32)
    k = np.asarray(k, np.float32)
    v = np.asarray(v, np.float32)
    Wq = np.asarray(Wq, np.float32)
    Wk = np.asarray(Wk, np.float32)
    Wv = np.asarray(Wv, np.float32)
    Wo = np.asarray(Wo, np.float32)
    bq = np.asarray(bq, np.float32)
    bk = np.asarray(bk, np.float32)
    bv = np.asarray(bv, np.float32)
    xts = {}
    for n in range(2):
        xts[n] = (np.ascontiguousarray(q[n].T).astype(bf),
                  np.ascontiguousarray(k[n].T).astype(bf),
                  np.ascontiguousarray(v[n].T).astype(bf))
    in_maps = []
    for c in range(8):
        n, hp = c // 4, c % 4
        sl = slice(P * hp, P * (hp + 1))
        xq, xk, xv = xts[n]
        in_maps.append({
            "xqt": xq, "xkt": xk, "xvt": xv,
            "wq": np.ascontiguousarray(Wq[:, sl]).astype(bf),
            "wk": np.ascontiguousarray(Wk[:, sl]).astype(bf),
            "wv": np.ascontiguousarray(Wv[:, sl]).astype(bf),
            "wo": np.ascontiguousarray(Wo[sl, :]),
            "bqs": (bq[sl] * 0.125).reshape(P, 1).astype(np.float32),
            "bkc": bk[sl].reshape(P, 1).astype(np.float32),
            "bvr": bv[sl].reshape(1, P).astype(bf),
        })
    return in_maps


def assemble(results, bo):
    bo = np.asarray(bo, np.float32)
    out = np.zeros((2, L, D), np.float32)
    for c in range(8):
        out[c // 4] += np.asarray(results[c]["out"], dtype=np.float32)
    out += bo[None, None, :]
    return out


def kernel(q, k, v, Wq, bq, Wk, bk, Wv, bv, Wo, bo):
    nc = get_nc()
    in_maps = make_in_maps(q, k, v, Wq, bq, Wk, bk, Wv, bv, Wo, bo)
    res = bass_utils.run_bass_kernel_spmd(nc, in_maps, core_ids=list(range(8)))
    return assemble(res.results, bo)


if __name__ == "__main__":
    build()
    print("build ok")
